# revision 32
# baseline (speedup 1.0000x reference)
"""Multi-head attention (B=4, N=2048, D=768, H=16) on 8 trn2 NeuronCores.

Sharding: core c = (batch b = c//2, head-group hg = c%2). Each core computes
attention for 8 heads of one batch element and the partial output projection
for those heads; the host sums the two partial projections per batch (the
tensor-parallel unshard) and adds the output bias.

Device kernel layout (per core) — matmul operands bf16, PSUM/softmax fp32:
  - All projections produce *transposed* activations: QT/KT [dim, seq] so the
    scores matmul S^T = K Q^T needs no transposes; softmax runs along q (free)
    with k on partitions; the row-sum for the softmax denominator is obtained
    by augmenting V with a ones column so the PV matmul emits it for free
    (rows 48/112 of the PSUM accumulator).
  - Heads are processed in pairs packed at partition offsets 0 and 64 so two
    K=48 (QK) / M=64 (PV) matmuls share the PE array via row/col tile groups.
  - exp() is split across engines per head: head A on the scalar engine
    (exact spline exp from PSUM), head B on the vector engine via a one-
    instruction Schraudolph approximation (S*A+B -> int16 whose bits are the
    bf16 exp; ~1.5% rms) so the per-kc softmax latency halves and neither
    engine paces the loop.
  - The PV matmul stream lags the scores stream by one kc so it never waits
    on a fresh exp.
  - Loop order qh-outer / pair-inner: the output projection accumulates the
    4 head-pairs in PSUM via matmul accumulation (no DVE add chain), then one
    DVE copy evicts each chunk for DMA.
  - Normalization reads the PV accumulator straight from PSUM: shuffle-
    broadcast the rowsums, approx-reciprocal, one full-width multiply (pad
    rows are exact zeros because V's pad columns are zero).
  - The next pairs' Q/K projection gens are interleaved into the attention
    loop to fill PE slack; tiny preamble matmuls warm each engine's semaphore
    clocks (walrus allows one wait per lowered instruction).
  - 1/sqrt(768) score scale is folded into WQ (and bQ) on the host.
"""

import math
import os

import numpy as np
import ml_dtypes

import concourse.bass as bass
import concourse.bacc as bacc
import concourse.tile as tile
from concourse import mybir
from concourse.bass_utils import run_bass_kernel_spmd
from contextlib import ExitStack

B, N, D, H, DH = 4, 2048, 768, 16, 48
P = 128
CC = D // P          # 6 contraction chunks of 128
KC = N // P          # 16 key chunks of 128
NPAIR = 4            # head pairs per core (8 heads)
QH = 1024            # q-half width (PSUM budget)
QB = 512             # matmul moving free dim (fp32 max)
F32 = mybir.dt.float32
BF16 = mybir.dt.bfloat16
I16 = mybir.dt.int16

# Schraudolph exp in bf16 bit space: bf16_bits(exp(x)) ~= x*SCH_A + SCH_B
# (then reinterpret the int16 as bf16).  SCH_A = 2^7/ln2; SCH_B centers the
# minimax relative error of the linear-mantissa approximation.
SCH_A = 184.6649652337873
SCH_B = 16245.0

_PROGRAM = None
LAST_RESULTS = None  # BassKernelResults of the most recent run (for test.py)


def _emit(ctx, tc, xt, wq, wk, wv, wp, bq, bk, bv, outt):
    nc = tc.nc
    Exp = mybir.ActivationFunctionType.Exp
    ADD = mybir.AluOpType.add
    MULT = mybir.AluOpType.mult

    consts = ctx.enter_context(tc.tile_pool(name="consts", bufs=1))
    qkvp = ctx.enter_context(tc.tile_pool(name="qkvp", bufs=1))
    vpool = ctx.enter_context(tc.tile_pool(name="vpool", bufs=1))
    ptp = ctx.enter_context(tc.tile_pool(name="ptp", bufs=3))
    ohp = ctx.enter_context(tc.tile_pool(name="ohp", bufs=2))
    rp = ctx.enter_context(tc.tile_pool(name="rp", bufs=1))
    otp = ctx.enter_context(tc.tile_pool(name="otp", bufs=2))
    sp = ctx.enter_context(tc.tile_pool(name="sp", bufs=1, space="PSUM"))
    ovp = ctx.enter_context(tc.tile_pool(name="ovp", bufs=1, space="PSUM"))
    pjp = ctx.enter_context(tc.tile_pool(name="pjp", bufs=2, space="PSUM"))

    # ---- constant loads ----
    wq_sb = consts.tile([P, NPAIR, CC, P], BF16)
    nc.sync.dma_start(out=wq_sb[:], in_=wq[:])
    wk_sb = consts.tile([P, NPAIR, CC, P], BF16)
    nc.sync.dma_start(out=wk_sb[:], in_=wk[:])
    bq_sb = consts.tile([P, NPAIR], F32)
    nc.sync.dma_start(out=bq_sb[:], in_=bq[:])
    bk_sb = consts.tile([P, NPAIR], F32)
    nc.sync.dma_start(out=bk_sb[:], in_=bk[:])
    xt_sb = consts.tile([P, CC, N], BF16)
    for h2 in range(2):
        for c in range(CC):
            nc.sync.dma_start(out=xt_sb[:, c, h2 * (N // 2):(h2 + 1) * (N // 2)],
                              in_=xt[:, c, h2 * (N // 2):(h2 + 1) * (N // 2)])
    wv_sb = consts.tile([P, CC, 8 * DH], BF16)
    nc.sync.dma_start(out=wv_sb[:], in_=wv[:])
    bv_sb = consts.tile([P, 8, DH], F32)
    nc.sync.dma_start(out=bv_sb[:], in_=bv[:])
    wp_sb = consts.tile([P, NPAIR, CC, P], BF16)
    nc.sync.dma_start(out=wp_sb[:], in_=wp[:])

    # ---- engine-clock warm-up ----
    # A self-loading fp32 matmul carries at most ONE semaphore wait in its
    # lowered form, so no real matmul may be the first observer of two new
    # semaphores.  Touch every DMA-loaded operand with a tiny dummy matmul
    # (PE) / copy (DVE) so each engine observes every DMA queue's semaphore
    # before real work begins.
    junk = pjp.tile([P, QB], F32, name="pj")
    for wi, ap in enumerate((
        wq_sb[0:1, 0, 0, 0:1], wk_sb[0:1, 0, 0, 0:1],
        xt_sb[0:1, 0, 0:1], xt_sb[0:1, 1, 0:1], xt_sb[0:1, 2, 0:1],
        xt_sb[0:1, 3, 0:1], xt_sb[0:1, 4, 0:1], xt_sb[0:1, 5, 0:1],
    )):
        nc.tensor.matmul(junk[0:1, wi:wi + 1], lhsT=ap, rhs=ap,
                         start=True, stop=True)
    # wv/wp warm-ups are deferred to just before their first real use so the
    # first QK-projection matmuls don't wait on the whole constant load.
    scr = otp.tile([P, 4], F32, name="scr")
    nc.vector.tensor_copy(scr[0:1, 0:1], bq_sb[0:1, 0:1])
    nc.vector.tensor_copy(scr[0:1, 1:2], bk_sb[0:1, 0:1])
    nc.vector.tensor_copy(scr[0:1, 2:3], bv_sb[0:1, 0, 0:1])

    # ---- Q/K projections (pair-packed transposed layout [128, 2048]) ----
    qt_all = [qkvp.tile([P, N], BF16, name=f"qt{p}") for p in range(NPAIR)]
    kt_all = [qkvp.tile([P, N], BF16, name=f"kt{p}") for p in range(NPAIR)]

    def emit_qk_gen(w_sb, b_sb, dst, pr, qb):
        ps = pjp.tile([P, QB], F32, name="pj")
        for c in range(CC):
            nc.tensor.matmul(
                ps[:],
                lhsT=w_sb[:, pr, c, :],
                rhs=xt_sb[:, c, qb * QB:(qb + 1) * QB],
                start=(c == 0),
                stop=(c == CC - 1),
            )
        nc.vector.tensor_scalar_add(
            dst[:, qb * QB:(qb + 1) * QB], ps[:], b_sb[:, pr:pr + 1]
        )

    # ---- V projection: [k-part, k-chunk, head, 64]: 48 dims | ones | zeros.
    # The ones column makes the PV matmul emit softmax row-sums at psum row
    # 48/112 for free; the zero pad makes PV write exact zeros to the pad
    # rows, so the normalization multiply can sweep all 128 rows.
    v_sb = vpool.tile([P, KC, 8, 64], BF16)
    nc.vector.memset(v_sb[:, :, :, DH:DH + 1], 1.0)
    nc.vector.memset(v_sb[:, :, :, DH + 1:64], 0.0)

    def emit_v_gen(s):
        ps = pjp.tile([P, QB], F32, name="pj")
        for c in range(CC):
            nc.tensor.matmul(
                ps[:, 0:8 * DH],
                lhsT=xt_sb[:, c, s * P:(s + 1) * P],
                rhs=wv_sb[:, c, :],
                start=(c == 0),
                stop=(c == CC - 1),
            )
        nc.vector.scalar_tensor_tensor(
            out=v_sb[:, s, :, 0:DH],
            in0=ps[:, 0:8 * DH].rearrange("p (h d) -> p h d", h=8),
            scalar=1.0,
            in1=bv_sb[:],
            op0=MULT,
            op1=ADD,
        )

    # pair 0 upfront; V chunks 0-3 upfront; the rest interleave into the
    # attention loop (fired from the per-pair prefetch lists) to fill PE
    # slack and keep the matmul-stream density smooth (the board's power
    # governor duty-cycles the PE when density stays high too long).
    for qb in range(4):
        emit_qk_gen(wq_sb, bq_sb, qt_all[0], 0, qb)
    for qb in range(4):
        emit_qk_gen(wk_sb, bk_sb, kt_all[0], 0, qb)

    def warm_wv():
        junkv = pjp.tile([P, QB], F32, name="pj")
        nc.tensor.matmul(junkv[0:1, 0:1], lhsT=wv_sb[0:1, 0, 0:1],
                         rhs=wv_sb[0:1, 0, 0:1], start=True, stop=True)

    warm_wv()
    for s in range(4):
        emit_v_gen(s)

    # prefetch schedule: pair p's two q-half blocks fire the remaining V
    # chunks (p0 only) then pair p+1's QK gens, ~1 per kc iteration.
    fire_lists = {p: {0: [], 1: []} for p in range(NPAIR)}
    for s in range(4, KC):
        fire_lists[0][0].append(lambda s=s: emit_v_gen(s))
    for pr in range(1, NPAIR):
        items = []
        for w_sb, b_sb, dsts in ((wq_sb, bq_sb, qt_all), (wk_sb, bk_sb, kt_all)):
            for qb in range(4):
                items.append(
                    lambda w=w_sb, b=b_sb, d=dsts[pr], pr=pr, qb=qb:
                        emit_qk_gen(w, b, d, pr, qb)
                )
        if pr == 1:
            # p0-qh0 is full with the V gens; all of pair 1's QK goes to qh1
            fire_lists[0][1] += items
        else:
            fire_lists[pr - 1][0] += items[:4]
            fire_lists[pr - 1][1] += items[4:]

    # persistent output staging [128, CC, N] fp32 is too big alongside pt;
    # stage per (e, qh) chunks and DMA immediately.
    outsb = ctx.enter_context(tc.tile_pool(name="outsb", bufs=4))

    m16 = [16] * 32

    def attention_block(p, qh, fires):
        """scores/exp/PV for head-pair p over q-half qh; returns oh tile."""
        q0 = qh * QH
        qt_cur, kt_cur = qt_all[p], kt_all[p]
        ov = ovp.tile([P, QH], F32, name="ov")
        # absorb fresh-tensor semaphore ticks (first observation of the DVE
        # eviction semaphores for this pair's qt/kt) into PE's clock
        junk2 = pjp.tile([P, QB], F32, name="pj")
        nc.tensor.matmul(junk2[0:1, 0:1], lhsT=qt_cur[0:1, 0:1],
                         rhs=qt_cur[0:1, 0:1], start=True, stop=True)
        nc.tensor.matmul(junk2[0:1, 1:2], lhsT=kt_cur[0:1, 0:1],
                         rhs=kt_cur[0:1, 0:1], start=True, stop=True)

        pts = {}

        def emit_scores(i):
            for hh in range(2):
                row0 = 64 * hh
                s_ps = sp.tile([P, QH], F32, name=f"s{hh}")
                for j in range(QH // QB):
                    nc.tensor.matmul(
                        s_ps[:, j * QB:(j + 1) * QB],
                        lhsT=kt_cur[row0:row0 + DH, i * P:(i + 1) * P],
                        rhs=qt_cur[row0:row0 + DH, q0 + j * QB:q0 + (j + 1) * QB],
                        start=True,
                        stop=True,
                        tile_position=(row0, 0),
                    )
                # exp split across engines: DVE takes head B on even kc via
                # the one-instruction Schraudolph approximation (~25% of all
                # scores), ACT the rest with the exact spline exp.
                if hh == 1 and (i % 8) in (0, 1, 2, 4, 6):
                    pt = ptp.tile([P, QH], I16, name="pt1")
                    nc.vector.tensor_scalar(
                        out=pt[:], in0=s_ps[:],
                        scalar1=SCH_A, scalar2=SCH_B,
                        op0=MULT, op1=ADD,
                    )
                else:
                    pt = ptp.tile([P, QH], BF16, name=f"pt0{hh}")
                    nc.scalar.activation(pt[:], s_ps[:], Exp)
                pts[(i, hh)] = pt

        def emit_pv(i):
            for hh in range(2):
                row0 = 64 * hh
                pt = pts.pop((i, hh))
                for j in range(QH // QB):
                    rhs = pt[:, j * QB:(j + 1) * QB]
                    if rhs.dtype == I16:
                        rhs = rhs.bitcast(BF16)
                    nc.tensor.matmul(
                        ov[row0:row0 + 64, j * QB:(j + 1) * QB],
                        lhsT=v_sb[:, i, 2 * p + hh, :],
                        rhs=rhs,
                        start=(i == 0),
                        stop=(i == KC - 1),
                        tile_position=(0, row0),
                        skip_group_check=True,
                    )

        # scores stream runs one kc ahead of the PV stream so PV never
        # waits on a fresh exp.
        emit_scores(0)
        for i in range(1, KC):
            emit_scores(i)
            emit_pv(i - 1)
            if fires:
                fires.pop(0)()
        emit_pv(KC - 1)

        # ---- normalization straight out of PSUM ----
        # rowsums sit at partition 48 (head A) / 112 (head B); shuffle-
        # broadcast them across each 64-row half, approx-reciprocal, then one
        # full-width multiply (pad rows of ov are exact zeros).
        oh = ohp.tile([P, QH], BF16, name=f"oh{p}")
        bc = rp.tile([P, QH], F32, name="bc")
        rec = rp.tile([P, QH], F32, name="rec")
        # per-512 halves: the first half of oh is ready for the output
        # projection before the second half's normalization runs
        for hq in range(2):
            hs = slice(hq * QB, (hq + 1) * QB)
            nc.vector.stream_shuffle(bc[0:32, hs], ov[32:64, hs], m16)
            nc.vector.stream_shuffle(bc[32:64, hs], ov[32:64, hs], m16)
            nc.vector.stream_shuffle(bc[64:96, hs], ov[96:P, hs], m16)
            nc.vector.stream_shuffle(bc[96:P, hs], ov[96:P, hs], m16)
            nc.vector.reciprocal_approx_fast(rec[:, hs], bc[:, hs])
            nc.vector.tensor_mul(oh[:, hs], ov[:, hs], rec[:, hs])
        return oh

    # p-outer: pair p runs both q-halves back to back (prefetch for pair
    # p+1 spread evenly across both), oh tiles for all pairs/q-halves are
    # kept, and the output projection for each q-half accumulates the 4
    # pairs in PSUM.  outproj(qh0) interleaves into block (p3, qh1)'s fire
    # slots; outproj(qh1) is the tail.
    oh_tiles = {}
    Copy = mybir.ActivationFunctionType.Copy

    def emit_outproj_chunk(qh, e):
        # qs-inner: each pair's weight LDWEIGHTS serves both 512-col q
        # slices; the two PSUM tiles accumulate the 4 pairs concurrently.
        ot = outsb.tile([P, QH], F32, name="ot")
        for qs in range(QH // QB):
            pj = pjp.tile([P, QB], F32, name="pj")
            for p in range(NPAIR):
                nc.tensor.matmul(
                    pj[:],
                    lhsT=wp_sb[:, p, e, :],
                    rhs=oh_tiles[(p, qh)][:, qs * QB:(qs + 1) * QB],
                    start=(p == 0),
                    stop=(p == NPAIR - 1),
                )
            # split the evictions across the two elementwise engines
            if qs == 0:
                nc.scalar.activation(ot[:, 0:QB], pj[:], Copy)
            else:
                nc.vector.tensor_copy(ot[:, QB:QH], pj[:])
        nc.sync.dma_start(out=outt[e, :, qh * QH:(qh + 1) * QH], in_=ot[:])

    def warm_wp():
        junkp = pjp.tile([P, QB], F32, name="pj")
        nc.tensor.matmul(junkp[0:1, 0:1], lhsT=wp_sb[0:1, 0, 0, 0:1],
                         rhs=wp_sb[0:1, 0, 0, 0:1], start=True, stop=True)

    fire_lists[3][1] = [warm_wp] + [
        lambda e=e: emit_outproj_chunk(0, e) for e in range(CC)
    ]

    for p in range(NPAIR):
        for qh in range(2):
            oh_tiles[(p, qh)] = attention_block(p, qh, fire_lists[p][qh])
            assert not fire_lists[p][qh], f"prefetch overflow {p},{qh}"

    for e in range(CC):
        emit_outproj_chunk(1, e)


def _build_program():
    # Bacc (not plain Bass): its compile pipeline legalizes semaphore waits
    # (move_matmul_waits_to_ldweights / generate_event_semaphores) for the
    # 1-wait-per-instruction TRN2 constraint walrus enforces.
    nc = bacc.Bacc(None)
    xt = nc.dram_tensor("xt", [P, CC, N], BF16, kind="ExternalInput")
    wq = nc.dram_tensor("wq", [P, NPAIR, CC, P], BF16, kind="ExternalInput")
    wk = nc.dram_tensor("wk", [P, NPAIR, CC, P], BF16, kind="ExternalInput")
    wv = nc.dram_tensor("wv", [P, CC, 8 * DH], BF16, kind="ExternalInput")
    wp = nc.dram_tensor("wp", [P, NPAIR, CC, P], BF16, kind="ExternalInput")
    bq = nc.dram_tensor("bq", [P, NPAIR], F32, kind="ExternalInput")
    bk = nc.dram_tensor("bk", [P, NPAIR], F32, kind="ExternalInput")
    bv = nc.dram_tensor("bv", [P, 8, DH], F32, kind="ExternalInput")
    outt = nc.dram_tensor("outt", [CC, P, N], F32, kind="ExternalOutput")
    with tile.TileContext(nc) as tc:
        with ExitStack() as ctx:
            _emit(ctx, tc, xt, wq, wk, wv, wp, bq, bk, bv, outt)
    nc.finalize()
    return nc


def _get_program():
    global _PROGRAM
    if _PROGRAM is None:
        _PROGRAM = _build_program()
    return _PROGRAM


def _bf16(a):
    return np.ascontiguousarray(a.astype(ml_dtypes.bfloat16))


def _pairize_cols(W, hg, scale=1.0):
    """[768, 768] -> [768, 512]: pair p gets head hg*8+2p at cols 0:48 and
    head hg*8+2p+1 at cols 64:112 of its 128-col block; the rest zeros."""
    Wp = np.zeros((D, 512), np.float32)
    for p in range(NPAIR):
        ha = (hg * 8 + 2 * p) * DH
        hb = (hg * 8 + 2 * p + 1) * DH
        Wp[:, p * P:p * P + DH] = W[:, ha:ha + DH]
        Wp[:, p * P + 64:p * P + 64 + DH] = W[:, hb:hb + DH]
    if scale != 1.0:
        Wp *= scale
    return Wp


def _pairize_bias(b, hg, scale=1.0):
    bp = np.zeros((P, NPAIR), np.float32)
    for p in range(NPAIR):
        ha = (hg * 8 + 2 * p) * DH
        hb = (hg * 8 + 2 * p + 1) * DH
        bp[0:DH, p] = b[ha:ha + DH]
        bp[64:64 + DH, p] = b[hb:hb + DH]
    if scale != 1.0:
        bp *= scale
    return bp


def _prep_inputs(inputs):
    x = np.asarray(inputs["x"], np.float32)
    WQ = np.asarray(inputs["WQ"], np.float32)
    WK = np.asarray(inputs["WK"], np.float32)
    WV = np.asarray(inputs["WV"], np.float32)
    WP = np.asarray(inputs["WP"], np.float32)
    bQ = np.asarray(inputs["bQ"], np.float32)
    bK = np.asarray(inputs["bK"], np.float32)
    bV = np.asarray(inputs["bV"], np.float32)
    scale = 1.0 / math.sqrt(D)

    per_hg = {}
    for hg in range(2):
        wq_d = _bf16(_pairize_cols(WQ, hg, scale).reshape(CC, P, NPAIR, P).transpose(1, 2, 0, 3))
        wk_d = _bf16(_pairize_cols(WK, hg).reshape(CC, P, NPAIR, P).transpose(1, 2, 0, 3))
        wv_d = _bf16(WV[:, hg * 384:(hg + 1) * 384].reshape(CC, P, 384).transpose(1, 0, 2))
        WPpad = np.zeros((NPAIR, P, D), np.float32)
        for p in range(NPAIR):
            ha = (hg * 8 + 2 * p) * DH
            hb = (hg * 8 + 2 * p + 1) * DH
            WPpad[p, 0:DH] = WP[ha:ha + DH, :]
            WPpad[p, 64:64 + DH] = WP[hb:hb + DH, :]
        wp_d = _bf16(WPpad.reshape(NPAIR, P, CC, P).transpose(1, 0, 2, 3))
        bq_d = _pairize_bias(bQ, hg, scale)
        bk_d = _pairize_bias(bK, hg)
        bv_d = np.ascontiguousarray(
            np.broadcast_to(bV[hg * 384:(hg + 1) * 384].reshape(8, DH), (P, 8, DH))
        )
        per_hg[hg] = dict(wq=wq_d, wk=wk_d, wv=wv_d, wp=wp_d, bq=bq_d, bk=bk_d, bv=bv_d)

    in_maps = []
    for c in range(8):
        b, hg = c // 2, c % 2
        xt_d = _bf16(x[b].T.reshape(CC, P, N).transpose(1, 0, 2))
        m = dict(per_hg[hg])
        m["xt"] = xt_d
        in_maps.append(m)
    return in_maps


def kernel(**inputs):
    global LAST_RESULTS
    bP = np.asarray(inputs["bP"], np.float32)
    nc = _get_program()
    in_maps = _prep_inputs(inputs)
    trace = bool(os.environ.get("BASS_KERNEL_TRACE"))
    tmpdir = os.environ.get("BASS_KERNEL_TMPDIR") or None
    res = run_bass_kernel_spmd(nc, in_maps, list(range(8)), trace=trace, tmpdir=tmpdir)
    LAST_RESULTS = res
    out = np.empty((B, N, D), np.float32)
    for b in range(B):
        t = res.results[2 * b]["outt"].reshape(D, N) + \
            res.results[2 * b + 1]["outt"].reshape(D, N)
        out[b] = t.T + bP
    return out


# revision 33
# speedup vs baseline: 1.0694x; 1.0694x over previous
"""Multi-head attention (B=4, N=2048, D=768, H=16) on 8 trn2 NeuronCores.

Sharding: core c = (batch b = c//2, head-group hg = c%2). Each core computes
attention for 8 heads of one batch element and the partial output projection
for those heads; the host sums the two partial projections per batch (the
tensor-parallel unshard) and adds the output bias.

Device kernel layout (per core) — matmul operands bf16, PSUM/softmax fp32:
  - All projections produce *transposed* activations: QT/KT [dim, seq] so the
    scores matmul S^T = K Q^T needs no transposes; softmax runs along q (free)
    with k on partitions; the row-sum for the softmax denominator is obtained
    by augmenting V with a ones column so the PV matmul emits it for free
    (rows 48/112 of the PSUM accumulator).
  - Heads are processed in pairs packed at partition offsets 0 and 64 so two
    K=48 (QK) / M=64 (PV) matmuls share the PE array via row/col tile groups.
  - exp() is split across engines per head: head A on the scalar engine
    (exact spline exp from PSUM), head B on the vector engine via a one-
    instruction Schraudolph approximation (S*A+B -> int16 whose bits are the
    bf16 exp; ~1.5% rms) so the per-kc softmax latency halves and neither
    engine paces the loop.
  - The PV matmul stream lags the scores stream by one kc so it never waits
    on a fresh exp.
  - Loop order qh-outer / pair-inner: the output projection accumulates the
    4 head-pairs in PSUM via matmul accumulation (no DVE add chain), then one
    DVE copy evicts each chunk for DMA.
  - Normalization reads the PV accumulator straight from PSUM: shuffle-
    broadcast the rowsums, approx-reciprocal, one full-width multiply (pad
    rows are exact zeros because V's pad columns are zero).
  - The next pairs' Q/K projection gens are interleaved into the attention
    loop to fill PE slack; tiny preamble matmuls warm each engine's semaphore
    clocks (walrus allows one wait per lowered instruction).
  - 1/sqrt(768) score scale is folded into WQ (and bQ) on the host.
"""

import math
import os

import numpy as np
import ml_dtypes

import concourse.bass as bass
import concourse.bacc as bacc
import concourse.tile as tile
from concourse import mybir
from concourse.bass_utils import run_bass_kernel_spmd
from contextlib import ExitStack

B, N, D, H, DH = 4, 2048, 768, 16, 48
P = 128
CC = D // P          # 6 contraction chunks of 128
KC = N // P          # 16 key chunks of 128
NPAIR = 4            # head pairs per core (8 heads)
QH = 1024            # q-half width (PSUM budget)
QB = 512             # matmul moving free dim (fp32 max)
F32 = mybir.dt.float32
BF16 = mybir.dt.bfloat16
I16 = mybir.dt.int16

# Schraudolph exp in bf16 bit space: bf16_bits(exp(x)) ~= x*SCH_A + SCH_B
# (then reinterpret the int16 as bf16).  SCH_A = 2^7/ln2; SCH_B centers the
# minimax relative error of the linear-mantissa approximation.
SCH_A = 184.6649652337873
SCH_B = 16245.0

_PROGRAM = None
LAST_RESULTS = None  # BassKernelResults of the most recent run (for test.py)


def _emit(ctx, tc, xt, wq, wk, wv, wp, bq, bk, bv, outt):
    nc = tc.nc
    Exp = mybir.ActivationFunctionType.Exp
    ADD = mybir.AluOpType.add
    MULT = mybir.AluOpType.mult

    consts = ctx.enter_context(tc.tile_pool(name="consts", bufs=1))
    qkvp = ctx.enter_context(tc.tile_pool(name="qkvp", bufs=1))
    vpool = ctx.enter_context(tc.tile_pool(name="vpool", bufs=1))
    ptp = ctx.enter_context(tc.tile_pool(name="ptp", bufs=4))
    ohp = ctx.enter_context(tc.tile_pool(name="ohp", bufs=2))
    rp = ctx.enter_context(tc.tile_pool(name="rp", bufs=1))
    otp = ctx.enter_context(tc.tile_pool(name="otp", bufs=2))
    sp = ctx.enter_context(tc.tile_pool(name="sp", bufs=1, space="PSUM"))
    ovp = ctx.enter_context(tc.tile_pool(name="ovp", bufs=1, space="PSUM"))
    pjp = ctx.enter_context(tc.tile_pool(name="pjp", bufs=2, space="PSUM"))

    # ---- constant loads ----
    wq_sb = consts.tile([P, NPAIR, CC, P], BF16)
    nc.sync.dma_start(out=wq_sb[:], in_=wq[:])
    wk_sb = consts.tile([P, NPAIR, CC, P], BF16)
    nc.sync.dma_start(out=wk_sb[:], in_=wk[:])
    bq_sb = consts.tile([P, NPAIR], F32)
    nc.sync.dma_start(out=bq_sb[:], in_=bq[:])
    bk_sb = consts.tile([P, NPAIR], F32)
    nc.sync.dma_start(out=bk_sb[:], in_=bk[:])
    xt_sb = consts.tile([P, CC, N], BF16)
    for h2 in range(2):
        for c in range(CC):
            nc.sync.dma_start(out=xt_sb[:, c, h2 * (N // 2):(h2 + 1) * (N // 2)],
                              in_=xt[:, c, h2 * (N // 2):(h2 + 1) * (N // 2)])
    wv_sb = consts.tile([P, CC, 8 * DH], BF16)
    nc.sync.dma_start(out=wv_sb[:], in_=wv[:])
    bv_sb = consts.tile([P, 8, DH], F32)
    nc.sync.dma_start(out=bv_sb[:], in_=bv[:])
    wp_sb = consts.tile([P, NPAIR, CC, P], BF16)
    nc.sync.dma_start(out=wp_sb[:], in_=wp[:])

    # ---- engine-clock warm-up ----
    # A self-loading fp32 matmul carries at most ONE semaphore wait in its
    # lowered form, so no real matmul may be the first observer of two new
    # semaphores.  Touch every DMA-loaded operand with a tiny dummy matmul
    # (PE) / copy (DVE) so each engine observes every DMA queue's semaphore
    # before real work begins.
    junk = pjp.tile([P, QB], F32, name="pj")
    for wi, ap in enumerate((
        wq_sb[0:1, 0, 0, 0:1], wk_sb[0:1, 0, 0, 0:1],
        xt_sb[0:1, 0, 0:1], xt_sb[0:1, 1, 0:1], xt_sb[0:1, 2, 0:1],
        xt_sb[0:1, 3, 0:1], xt_sb[0:1, 4, 0:1], xt_sb[0:1, 5, 0:1],
    )):
        nc.tensor.matmul(junk[0:1, wi:wi + 1], lhsT=ap, rhs=ap,
                         start=True, stop=True)
    # wv/wp warm-ups are deferred to just before their first real use so the
    # first QK-projection matmuls don't wait on the whole constant load.
    scr = otp.tile([P, 4], F32, name="scr")
    nc.vector.tensor_copy(scr[0:1, 0:1], bq_sb[0:1, 0:1])
    nc.vector.tensor_copy(scr[0:1, 1:2], bk_sb[0:1, 0:1])
    nc.vector.tensor_copy(scr[0:1, 2:3], bv_sb[0:1, 0, 0:1])

    # ---- Q/K projections (pair-packed transposed layout [128, 2048]) ----
    qt_all = [qkvp.tile([P, N], BF16, name=f"qt{p}") for p in range(NPAIR)]
    kt_all = [qkvp.tile([P, N], BF16, name=f"kt{p}") for p in range(NPAIR)]

    def emit_qk_gen(w_sb, b_sb, dst, pr, qb):
        ps = pjp.tile([P, QB], F32, name="pj")
        for c in range(CC):
            nc.tensor.matmul(
                ps[:],
                lhsT=w_sb[:, pr, c, :],
                rhs=xt_sb[:, c, qb * QB:(qb + 1) * QB],
                start=(c == 0),
                stop=(c == CC - 1),
            )
        nc.vector.tensor_scalar_add(
            dst[:, qb * QB:(qb + 1) * QB], ps[:], b_sb[:, pr:pr + 1]
        )

    # ---- V projection: [k-part, k-chunk, head, 64]: 48 dims | ones | zeros.
    # The ones column makes the PV matmul emit softmax row-sums at psum row
    # 48/112 for free; the zero pad makes PV write exact zeros to the pad
    # rows, so the normalization multiply can sweep all 128 rows.
    v_sb = vpool.tile([P, KC, 8, 64], BF16)
    nc.vector.memset(v_sb[:, :, :, DH:DH + 1], 1.0)
    nc.vector.memset(v_sb[:, :, :, DH + 1:64], 0.0)

    def emit_v_gen(s):
        ps = pjp.tile([P, QB], F32, name="pj")
        for c in range(CC):
            nc.tensor.matmul(
                ps[:, 0:8 * DH],
                lhsT=xt_sb[:, c, s * P:(s + 1) * P],
                rhs=wv_sb[:, c, :],
                start=(c == 0),
                stop=(c == CC - 1),
            )
        nc.vector.scalar_tensor_tensor(
            out=v_sb[:, s, :, 0:DH],
            in0=ps[:, 0:8 * DH].rearrange("p (h d) -> p h d", h=8),
            scalar=1.0,
            in1=bv_sb[:],
            op0=MULT,
            op1=ADD,
        )

    # pair 0 upfront; V chunks 0-3 upfront; the rest interleave into the
    # attention loop (fired from the per-pair prefetch lists) to fill PE
    # slack and keep the matmul-stream density smooth (the board's power
    # governor duty-cycles the PE when density stays high too long).
    for qb in range(4):
        emit_qk_gen(wq_sb, bq_sb, qt_all[0], 0, qb)
    for qb in range(4):
        emit_qk_gen(wk_sb, bk_sb, kt_all[0], 0, qb)

    def warm_wv():
        junkv = pjp.tile([P, QB], F32, name="pj")
        nc.tensor.matmul(junkv[0:1, 0:1], lhsT=wv_sb[0:1, 0, 0:1],
                         rhs=wv_sb[0:1, 0, 0:1], start=True, stop=True)

    warm_wv()
    for s in range(4):
        emit_v_gen(s)

    # prefetch schedule: pair p's two q-half blocks fire the remaining V
    # chunks (p0 only) then pair p+1's QK gens, ~1 per kc iteration.
    fire_lists = {p: {0: [], 1: []} for p in range(NPAIR)}
    for s in range(4, KC):
        fire_lists[0][0].append(lambda s=s: emit_v_gen(s))
    for pr in range(1, NPAIR):
        items = []
        for w_sb, b_sb, dsts in ((wq_sb, bq_sb, qt_all), (wk_sb, bk_sb, kt_all)):
            for qb in range(4):
                items.append(
                    lambda w=w_sb, b=b_sb, d=dsts[pr], pr=pr, qb=qb:
                        emit_qk_gen(w, b, d, pr, qb)
                )
        if pr == 1:
            # p0-qh0 is full with the V gens; all of pair 1's QK goes to qh1
            fire_lists[0][1] += items
        else:
            fire_lists[pr - 1][0] += items[:4]
            fire_lists[pr - 1][1] += items[4:]

    # persistent output staging [128, CC, N] fp32 is too big alongside pt;
    # stage per (e, qh) chunks and DMA immediately.
    outsb = ctx.enter_context(tc.tile_pool(name="outsb", bufs=4))

    m16 = [16] * 32

    def attention_block(p, qh, fires):
        """scores/exp/PV for head-pair p over q-half qh; returns oh tile."""
        q0 = qh * QH
        qt_cur, kt_cur = qt_all[p], kt_all[p]
        ov = ovp.tile([P, QH], F32, name="ov")
        # absorb fresh-tensor semaphore ticks (first observation of the DVE
        # eviction semaphores for this pair's qt/kt) into PE's clock
        junk2 = pjp.tile([P, QB], F32, name="pj")
        nc.tensor.matmul(junk2[0:1, 0:1], lhsT=qt_cur[0:1, 0:1],
                         rhs=qt_cur[0:1, 0:1], start=True, stop=True)
        nc.tensor.matmul(junk2[0:1, 1:2], lhsT=kt_cur[0:1, 0:1],
                         rhs=kt_cur[0:1, 0:1], start=True, stop=True)

        pts = {}

        def emit_scores(i):
            for hh in range(2):
                row0 = 64 * hh
                s_ps = sp.tile([P, QH], F32, name=f"s{hh}")
                for j in range(QH // QB):
                    nc.tensor.matmul(
                        s_ps[:, j * QB:(j + 1) * QB],
                        lhsT=kt_cur[row0:row0 + DH, i * P:(i + 1) * P],
                        rhs=qt_cur[row0:row0 + DH, q0 + j * QB:q0 + (j + 1) * QB],
                        start=True,
                        stop=True,
                        tile_position=(row0, 0),
                    )
                # exp split across engines: DVE takes head B on even kc via
                # the one-instruction Schraudolph approximation (~25% of all
                # scores), ACT the rest with the exact spline exp.
                if hh == 1 and i % 2 == 0:
                    pt = ptp.tile([P, QH], I16, name="pt1")
                    nc.vector.tensor_scalar(
                        out=pt[:], in0=s_ps[:],
                        scalar1=SCH_A, scalar2=SCH_B,
                        op0=MULT, op1=ADD,
                    )
                else:
                    pt = ptp.tile([P, QH], BF16, name=f"pt0{hh}")
                    nc.scalar.activation(pt[:], s_ps[:], Exp)
                pts[(i, hh)] = pt

        def emit_pv(i):
            for hh in range(2):
                row0 = 64 * hh
                pt = pts.pop((i, hh))
                for j in range(QH // QB):
                    rhs = pt[:, j * QB:(j + 1) * QB]
                    if rhs.dtype == I16:
                        rhs = rhs.bitcast(BF16)
                    nc.tensor.matmul(
                        ov[row0:row0 + 64, j * QB:(j + 1) * QB],
                        lhsT=v_sb[:, i, 2 * p + hh, :],
                        rhs=rhs,
                        start=(i == 0),
                        stop=(i == KC - 1),
                        tile_position=(0, row0),
                        skip_group_check=True,
                    )

        # scores stream runs one kc ahead of the PV stream so PV never
        # waits on a fresh exp.
        emit_scores(0)
        for i in range(1, KC):
            emit_scores(i)
            emit_pv(i - 1)
            if fires:
                fires.pop(0)()
        emit_pv(KC - 1)

        # ---- normalization straight out of PSUM ----
        # rowsums sit at partition 48 (head A) / 112 (head B); shuffle-
        # broadcast them across each 64-row half, approx-reciprocal, then one
        # full-width multiply (pad rows of ov are exact zeros).
        oh = ohp.tile([P, QH], BF16, name=f"oh{p}")
        bc = rp.tile([P, QH], F32, name="bc")
        rec = rp.tile([P, QH], F32, name="rec")
        nc.vector.stream_shuffle(bc[0:32, :], ov[32:64, :], m16)
        nc.vector.stream_shuffle(bc[32:64, :], ov[32:64, :], m16)
        nc.vector.stream_shuffle(bc[64:96, :], ov[96:P, :], m16)
        nc.vector.stream_shuffle(bc[96:P, :], ov[96:P, :], m16)
        nc.vector.reciprocal_approx_fast(rec[:], bc[:])
        nc.vector.tensor_mul(oh[:], ov[:], rec[:])
        return oh

    # p-outer: pair p runs both q-halves back to back (prefetch for pair
    # p+1 spread evenly across both), oh tiles for all pairs/q-halves are
    # kept, and the output projection for each q-half accumulates the 4
    # pairs in PSUM.  outproj(qh0) interleaves into block (p3, qh1)'s fire
    # slots; outproj(qh1) is the tail.
    oh_tiles = {}
    Copy = mybir.ActivationFunctionType.Copy

    def emit_outproj_chunk(qh, e):
        # qs-inner: each pair's weight LDWEIGHTS serves both 512-col q
        # slices; the two PSUM tiles accumulate the 4 pairs concurrently.
        ot = outsb.tile([P, QH], F32, name="ot")
        for qs in range(QH // QB):
            pj = pjp.tile([P, QB], F32, name="pj")
            for p in range(NPAIR):
                nc.tensor.matmul(
                    pj[:],
                    lhsT=wp_sb[:, p, e, :],
                    rhs=oh_tiles[(p, qh)][:, qs * QB:(qs + 1) * QB],
                    start=(p == 0),
                    stop=(p == NPAIR - 1),
                )
            # split the evictions across the two elementwise engines
            if qs == 0:
                nc.scalar.activation(ot[:, 0:QB], pj[:], Copy)
            else:
                nc.vector.tensor_copy(ot[:, QB:QH], pj[:])
        nc.sync.dma_start(out=outt[e, :, qh * QH:(qh + 1) * QH], in_=ot[:])

    def warm_wp():
        junkp = pjp.tile([P, QB], F32, name="pj")
        nc.tensor.matmul(junkp[0:1, 0:1], lhsT=wp_sb[0:1, 0, 0, 0:1],
                         rhs=wp_sb[0:1, 0, 0, 0:1], start=True, stop=True)

    fire_lists[3][1] = [warm_wp] + [
        lambda e=e: emit_outproj_chunk(0, e) for e in range(CC)
    ]

    for p in range(NPAIR):
        for qh in range(2):
            oh_tiles[(p, qh)] = attention_block(p, qh, fire_lists[p][qh])
            assert not fire_lists[p][qh], f"prefetch overflow {p},{qh}"

    for e in range(CC):
        emit_outproj_chunk(1, e)


def _build_program():
    # Bacc (not plain Bass): its compile pipeline legalizes semaphore waits
    # (move_matmul_waits_to_ldweights / generate_event_semaphores) for the
    # 1-wait-per-instruction TRN2 constraint walrus enforces.
    nc = bacc.Bacc(None)
    xt = nc.dram_tensor("xt", [P, CC, N], BF16, kind="ExternalInput")
    wq = nc.dram_tensor("wq", [P, NPAIR, CC, P], BF16, kind="ExternalInput")
    wk = nc.dram_tensor("wk", [P, NPAIR, CC, P], BF16, kind="ExternalInput")
    wv = nc.dram_tensor("wv", [P, CC, 8 * DH], BF16, kind="ExternalInput")
    wp = nc.dram_tensor("wp", [P, NPAIR, CC, P], BF16, kind="ExternalInput")
    bq = nc.dram_tensor("bq", [P, NPAIR], F32, kind="ExternalInput")
    bk = nc.dram_tensor("bk", [P, NPAIR], F32, kind="ExternalInput")
    bv = nc.dram_tensor("bv", [P, 8, DH], F32, kind="ExternalInput")
    outt = nc.dram_tensor("outt", [CC, P, N], F32, kind="ExternalOutput")
    with tile.TileContext(nc) as tc:
        with ExitStack() as ctx:
            _emit(ctx, tc, xt, wq, wk, wv, wp, bq, bk, bv, outt)
    nc.finalize()
    return nc


def _get_program():
    global _PROGRAM
    if _PROGRAM is None:
        _PROGRAM = _build_program()
    return _PROGRAM


def _bf16(a):
    return np.ascontiguousarray(a.astype(ml_dtypes.bfloat16))


def _pairize_cols(W, hg, scale=1.0):
    """[768, 768] -> [768, 512]: pair p gets head hg*8+2p at cols 0:48 and
    head hg*8+2p+1 at cols 64:112 of its 128-col block; the rest zeros."""
    Wp = np.zeros((D, 512), np.float32)
    for p in range(NPAIR):
        ha = (hg * 8 + 2 * p) * DH
        hb = (hg * 8 + 2 * p + 1) * DH
        Wp[:, p * P:p * P + DH] = W[:, ha:ha + DH]
        Wp[:, p * P + 64:p * P + 64 + DH] = W[:, hb:hb + DH]
    if scale != 1.0:
        Wp *= scale
    return Wp


def _pairize_bias(b, hg, scale=1.0):
    bp = np.zeros((P, NPAIR), np.float32)
    for p in range(NPAIR):
        ha = (hg * 8 + 2 * p) * DH
        hb = (hg * 8 + 2 * p + 1) * DH
        bp[0:DH, p] = b[ha:ha + DH]
        bp[64:64 + DH, p] = b[hb:hb + DH]
    if scale != 1.0:
        bp *= scale
    return bp


def _prep_inputs(inputs):
    x = np.asarray(inputs["x"], np.float32)
    WQ = np.asarray(inputs["WQ"], np.float32)
    WK = np.asarray(inputs["WK"], np.float32)
    WV = np.asarray(inputs["WV"], np.float32)
    WP = np.asarray(inputs["WP"], np.float32)
    bQ = np.asarray(inputs["bQ"], np.float32)
    bK = np.asarray(inputs["bK"], np.float32)
    bV = np.asarray(inputs["bV"], np.float32)
    scale = 1.0 / math.sqrt(D)

    per_hg = {}
    for hg in range(2):
        wq_d = _bf16(_pairize_cols(WQ, hg, scale).reshape(CC, P, NPAIR, P).transpose(1, 2, 0, 3))
        wk_d = _bf16(_pairize_cols(WK, hg).reshape(CC, P, NPAIR, P).transpose(1, 2, 0, 3))
        wv_d = _bf16(WV[:, hg * 384:(hg + 1) * 384].reshape(CC, P, 384).transpose(1, 0, 2))
        WPpad = np.zeros((NPAIR, P, D), np.float32)
        for p in range(NPAIR):
            ha = (hg * 8 + 2 * p) * DH
            hb = (hg * 8 + 2 * p + 1) * DH
            WPpad[p, 0:DH] = WP[ha:ha + DH, :]
            WPpad[p, 64:64 + DH] = WP[hb:hb + DH, :]
        wp_d = _bf16(WPpad.reshape(NPAIR, P, CC, P).transpose(1, 0, 2, 3))
        bq_d = _pairize_bias(bQ, hg, scale)
        bk_d = _pairize_bias(bK, hg)
        bv_d = np.ascontiguousarray(
            np.broadcast_to(bV[hg * 384:(hg + 1) * 384].reshape(8, DH), (P, 8, DH))
        )
        per_hg[hg] = dict(wq=wq_d, wk=wk_d, wv=wv_d, wp=wp_d, bq=bq_d, bk=bk_d, bv=bv_d)

    in_maps = []
    for c in range(8):
        b, hg = c // 2, c % 2
        xt_d = _bf16(x[b].T.reshape(CC, P, N).transpose(1, 0, 2))
        m = dict(per_hg[hg])
        m["xt"] = xt_d
        in_maps.append(m)
    return in_maps


def kernel(**inputs):
    global LAST_RESULTS
    bP = np.asarray(inputs["bP"], np.float32)
    nc = _get_program()
    in_maps = _prep_inputs(inputs)
    trace = bool(os.environ.get("BASS_KERNEL_TRACE"))
    tmpdir = os.environ.get("BASS_KERNEL_TMPDIR") or None
    res = run_bass_kernel_spmd(nc, in_maps, list(range(8)), trace=trace, tmpdir=tmpdir)
    LAST_RESULTS = res
    out = np.empty((B, N, D), np.float32)
    for b in range(B):
        t = res.results[2 * b]["outt"].reshape(D, N) + \
            res.results[2 * b + 1]["outt"].reshape(D, N)
        out[b] = t.T + bP
    return out


# revision 34
# speedup vs baseline: 1.2696x; 1.1872x over previous
"""Multi-head attention (B=4, N=2048, D=768, H=16) on 8 trn2 NeuronCores.

Sharding: core c = (batch b = c//2, head-group hg = c%2). Each core computes
attention for 8 heads of one batch element and the partial output projection
for those heads; the host sums the two partial projections per batch (the
tensor-parallel unshard) and adds the output bias.

Device kernel layout (per core) — matmul operands bf16, PSUM/softmax fp32:
  - All projections produce *transposed* activations: QT/KT [dim, seq] so the
    scores matmul S^T = K Q^T needs no transposes; softmax runs along q (free)
    with k on partitions; the row-sum for the softmax denominator is obtained
    by augmenting V with a ones column so the PV matmul emits it for free
    (rows 48/112 of the PSUM accumulator).
  - Heads are processed in pairs packed at partition offsets 0 and 64 so two
    K=48 (QK) / M=64 (PV) matmuls share the PE array via row/col tile groups.
  - exp() is split across engines per head: head A on the scalar engine
    (exact spline exp from PSUM), head B on the vector engine via a one-
    instruction Schraudolph approximation (S*A+B -> int16 whose bits are the
    bf16 exp; ~1.5% rms) so the per-kc softmax latency halves and neither
    engine paces the loop.
  - The PV matmul stream lags the scores stream by one kc so it never waits
    on a fresh exp.
  - Loop order qh-outer / pair-inner: the output projection accumulates the
    4 head-pairs in PSUM via matmul accumulation (no DVE add chain), then one
    DVE copy evicts each chunk for DMA.
  - Normalization reads the PV accumulator straight from PSUM: shuffle-
    broadcast the rowsums, approx-reciprocal, one full-width multiply (pad
    rows are exact zeros because V's pad columns are zero).
  - The next pairs' Q/K projection gens are interleaved into the attention
    loop to fill PE slack; tiny preamble matmuls warm each engine's semaphore
    clocks (walrus allows one wait per lowered instruction).
  - 1/sqrt(768) score scale is folded into WQ (and bQ) on the host.
"""

import math
import os

import numpy as np
import ml_dtypes

import concourse.bass as bass
import concourse.bacc as bacc
import concourse.tile as tile
from concourse import mybir
from concourse.bass_utils import run_bass_kernel_spmd
from contextlib import ExitStack

B, N, D, H, DH = 4, 2048, 768, 16, 48
P = 128
CC = D // P          # 6 contraction chunks of 128
KC = N // P          # 16 key chunks of 128
NPAIR = 4            # head pairs per core (8 heads)
QH = 1024            # q-half width (PSUM budget)
QB = 512             # matmul moving free dim (fp32 max)
F32 = mybir.dt.float32
BF16 = mybir.dt.bfloat16
I16 = mybir.dt.int16

# Schraudolph exp in bf16 bit space: bf16_bits(exp(x)) ~= x*SCH_A + SCH_B
# (then reinterpret the int16 as bf16).  SCH_A = 2^7/ln2; SCH_B centers the
# minimax relative error of the linear-mantissa approximation.
SCH_A = 184.6649652337873
SCH_B = 16245.0

_PROGRAM = None
LAST_RESULTS = None  # BassKernelResults of the most recent run (for test.py)


def _emit(ctx, tc, xt, wq, wk, wv, wp, bq, bk, bv, outt):
    nc = tc.nc
    Exp = mybir.ActivationFunctionType.Exp
    ADD = mybir.AluOpType.add
    MULT = mybir.AluOpType.mult

    consts = ctx.enter_context(tc.tile_pool(name="consts", bufs=1))
    qkvp = ctx.enter_context(tc.tile_pool(name="qkvp", bufs=1))
    vpool = ctx.enter_context(tc.tile_pool(name="vpool", bufs=1))
    ptp = ctx.enter_context(tc.tile_pool(name="ptp", bufs=3))
    ohp = ctx.enter_context(tc.tile_pool(name="ohp", bufs=2))
    rp = ctx.enter_context(tc.tile_pool(name="rp", bufs=1))
    otp = ctx.enter_context(tc.tile_pool(name="otp", bufs=2))
    sp = ctx.enter_context(tc.tile_pool(name="sp", bufs=1, space="PSUM"))
    ovp = ctx.enter_context(tc.tile_pool(name="ovp", bufs=1, space="PSUM"))
    pjp = ctx.enter_context(tc.tile_pool(name="pjp", bufs=2, space="PSUM"))

    # ---- constant loads ----
    wq_sb = consts.tile([P, NPAIR, CC, P], BF16)
    nc.sync.dma_start(out=wq_sb[:], in_=wq[:])
    wk_sb = consts.tile([P, NPAIR, CC, P], BF16)
    nc.sync.dma_start(out=wk_sb[:], in_=wk[:])
    bq_sb = consts.tile([P, NPAIR], F32)
    nc.sync.dma_start(out=bq_sb[:], in_=bq[:])
    bk_sb = consts.tile([P, NPAIR], F32)
    nc.sync.dma_start(out=bk_sb[:], in_=bk[:])
    xt_sb = consts.tile([P, CC, N], BF16)
    for h2 in range(2):
        for c in range(CC):
            nc.sync.dma_start(out=xt_sb[:, c, h2 * (N // 2):(h2 + 1) * (N // 2)],
                              in_=xt[:, c, h2 * (N // 2):(h2 + 1) * (N // 2)])
    wv_sb = consts.tile([P, CC, 8 * DH], BF16)
    nc.sync.dma_start(out=wv_sb[:], in_=wv[:])
    bv_sb = consts.tile([P, 8, DH], F32)
    nc.sync.dma_start(out=bv_sb[:], in_=bv[:])
    wp_sb = consts.tile([P, NPAIR, CC, P], BF16)
    nc.sync.dma_start(out=wp_sb[:], in_=wp[:])

    # ---- engine-clock warm-up ----
    # A self-loading fp32 matmul carries at most ONE semaphore wait in its
    # lowered form, so no real matmul may be the first observer of two new
    # semaphores.  Touch every DMA-loaded operand with a tiny dummy matmul
    # (PE) / copy (DVE) so each engine observes every DMA queue's semaphore
    # before real work begins.
    junk = pjp.tile([P, QB], F32, name="pj")
    for wi, ap in enumerate((
        wq_sb[0:1, 0, 0, 0:1], wk_sb[0:1, 0, 0, 0:1],
        xt_sb[0:1, 0, 0:1], xt_sb[0:1, 1, 0:1], xt_sb[0:1, 2, 0:1],
        xt_sb[0:1, 3, 0:1], xt_sb[0:1, 4, 0:1], xt_sb[0:1, 5, 0:1],
    )):
        nc.tensor.matmul(junk[0:1, wi:wi + 1], lhsT=ap, rhs=ap,
                         start=True, stop=True)
    # wv/wp warm-ups are deferred to just before their first real use so the
    # first QK-projection matmuls don't wait on the whole constant load.
    scr = otp.tile([P, 4], F32, name="scr")
    nc.vector.tensor_copy(scr[0:1, 0:1], bq_sb[0:1, 0:1])
    nc.vector.tensor_copy(scr[0:1, 1:2], bk_sb[0:1, 0:1])
    nc.vector.tensor_copy(scr[0:1, 2:3], bv_sb[0:1, 0, 0:1])

    # ---- Q/K projections (pair-packed transposed layout [128, 2048]) ----
    qt_all = [qkvp.tile([P, N], BF16, name=f"qt{p}") for p in range(NPAIR)]
    kt_all = [qkvp.tile([P, N], BF16, name=f"kt{p}") for p in range(NPAIR)]

    def emit_qk_gen(w_sb, b_sb, dst, pr, qb):
        ps = pjp.tile([P, QB], F32, name="pj")
        for c in range(CC):
            nc.tensor.matmul(
                ps[:],
                lhsT=w_sb[:, pr, c, :],
                rhs=xt_sb[:, c, qb * QB:(qb + 1) * QB],
                start=(c == 0),
                stop=(c == CC - 1),
            )
        nc.vector.tensor_scalar_add(
            dst[:, qb * QB:(qb + 1) * QB], ps[:], b_sb[:, pr:pr + 1]
        )

    # ---- V projection: [k-part, k-chunk, head, 64]: 48 dims | ones | zeros.
    # The ones column makes the PV matmul emit softmax row-sums at psum row
    # 48/112 for free; the zero pad makes PV write exact zeros to the pad
    # rows, so the normalization multiply can sweep all 128 rows.
    v_sb = vpool.tile([P, KC, 8, 64], BF16)
    nc.vector.memset(v_sb[:, :, :, DH:DH + 1], 1.0)
    nc.vector.memset(v_sb[:, :, :, DH + 1:64], 0.0)

    def emit_v_gen(s):
        ps = pjp.tile([P, QB], F32, name="pj")
        for c in range(CC):
            nc.tensor.matmul(
                ps[:, 0:8 * DH],
                lhsT=xt_sb[:, c, s * P:(s + 1) * P],
                rhs=wv_sb[:, c, :],
                start=(c == 0),
                stop=(c == CC - 1),
            )
        nc.vector.scalar_tensor_tensor(
            out=v_sb[:, s, :, 0:DH],
            in0=ps[:, 0:8 * DH].rearrange("p (h d) -> p h d", h=8),
            scalar=1.0,
            in1=bv_sb[:],
            op0=MULT,
            op1=ADD,
        )

    # pair 0 upfront; V chunks 0-3 upfront; the rest interleave into the
    # attention loop (fired from the per-pair prefetch lists) to fill PE
    # slack and keep the matmul-stream density smooth (the board's power
    # governor duty-cycles the PE when density stays high too long).
    for qb in range(4):
        emit_qk_gen(wq_sb, bq_sb, qt_all[0], 0, qb)
    for qb in range(4):
        emit_qk_gen(wk_sb, bk_sb, kt_all[0], 0, qb)

    def warm_wv():
        junkv = pjp.tile([P, QB], F32, name="pj")
        nc.tensor.matmul(junkv[0:1, 0:1], lhsT=wv_sb[0:1, 0, 0:1],
                         rhs=wv_sb[0:1, 0, 0:1], start=True, stop=True)

    warm_wv()
    for s in range(4):
        emit_v_gen(s)

    # prefetch schedule: pair p's two q-half blocks fire the remaining V
    # chunks (p0 only) then pair p+1's QK gens, ~1 per kc iteration.
    fire_lists = {p: {0: [], 1: []} for p in range(NPAIR)}
    for s in range(4, KC):
        fire_lists[0][0].append(lambda s=s: emit_v_gen(s))
    for pr in range(1, NPAIR):
        items = []
        for w_sb, b_sb, dsts in ((wq_sb, bq_sb, qt_all), (wk_sb, bk_sb, kt_all)):
            for qb in range(4):
                items.append(
                    lambda w=w_sb, b=b_sb, d=dsts[pr], pr=pr, qb=qb:
                        emit_qk_gen(w, b, d, pr, qb)
                )
        if pr == 1:
            # p0-qh0 is full with the V gens; all of pair 1's QK goes to qh1
            fire_lists[0][1] += items
        else:
            fire_lists[pr - 1][0] += items[:4]
            fire_lists[pr - 1][1] += items[4:]

    # persistent output staging [128, CC, N] fp32 is too big alongside pt;
    # stage per (e, qh) chunks and DMA immediately.
    outsb = ctx.enter_context(tc.tile_pool(name="outsb", bufs=4))

    m16 = [16] * 32

    def attention_block(p, qh, fires):
        """scores/exp/PV for head-pair p over q-half qh; returns oh tile."""
        q0 = qh * QH
        qt_cur, kt_cur = qt_all[p], kt_all[p]
        ov = ovp.tile([P, QH], F32, name="ov")
        # absorb fresh-tensor semaphore ticks (first observation of the DVE
        # eviction semaphores for this pair's qt/kt) into PE's clock
        junk2 = pjp.tile([P, QB], F32, name="pj")
        nc.tensor.matmul(junk2[0:1, 0:1], lhsT=qt_cur[0:1, 0:1],
                         rhs=qt_cur[0:1, 0:1], start=True, stop=True)
        nc.tensor.matmul(junk2[0:1, 1:2], lhsT=kt_cur[0:1, 0:1],
                         rhs=kt_cur[0:1, 0:1], start=True, stop=True)

        pts = {}

        def emit_scores(i):
            for hh in range(2):
                row0 = 64 * hh
                s_ps = sp.tile([P, QH], F32, name=f"s{hh}")
                for j in range(QH // QB):
                    nc.tensor.matmul(
                        s_ps[:, j * QB:(j + 1) * QB],
                        lhsT=kt_cur[row0:row0 + DH, i * P:(i + 1) * P],
                        rhs=qt_cur[row0:row0 + DH, q0 + j * QB:q0 + (j + 1) * QB],
                        start=True,
                        stop=True,
                        tile_position=(row0, 0),
                    )
                # exp split across engines: DVE takes head B on even kc via
                # the one-instruction Schraudolph approximation (~25% of all
                # scores), ACT the rest with the exact spline exp.
                if hh == 1 and i % 2 == 0:
                    pt = ptp.tile([P, QH], I16, name="pt1")
                    nc.vector.tensor_scalar(
                        out=pt[:], in0=s_ps[:],
                        scalar1=SCH_A, scalar2=SCH_B,
                        op0=MULT, op1=ADD,
                    )
                else:
                    pt = ptp.tile([P, QH], BF16, name=f"pt0{hh}")
                    nc.scalar.activation(pt[:], s_ps[:], Exp)
                pts[(i, hh)] = pt

        def emit_pv(i):
            for hh in range(2):
                row0 = 64 * hh
                pt = pts.pop((i, hh))
                for j in range(QH // QB):
                    rhs = pt[:, j * QB:(j + 1) * QB]
                    if rhs.dtype == I16:
                        rhs = rhs.bitcast(BF16)
                    nc.tensor.matmul(
                        ov[row0:row0 + 64, j * QB:(j + 1) * QB],
                        lhsT=v_sb[:, i, 2 * p + hh, :],
                        rhs=rhs,
                        start=(i == 0),
                        stop=(i == KC - 1),
                        tile_position=(0, row0),
                        skip_group_check=True,
                    )

        # scores stream runs one kc ahead of the PV stream so PV never
        # waits on a fresh exp.
        emit_scores(0)
        for i in range(1, KC):
            emit_scores(i)
            emit_pv(i - 1)
            if fires:
                fires.pop(0)()
        emit_pv(KC - 1)

        # ---- normalization straight out of PSUM ----
        # rowsums sit at partition 48 (head A) / 112 (head B); shuffle-
        # broadcast them across each 64-row half, approx-reciprocal, then one
        # full-width multiply (pad rows of ov are exact zeros).
        oh = ohp.tile([P, QH], BF16, name=f"oh{p}")
        bc = rp.tile([P, QH], F32, name="bc")
        rec = rp.tile([P, QH], F32, name="rec")
        nc.vector.stream_shuffle(bc[0:32, :], ov[32:64, :], m16)
        nc.vector.stream_shuffle(bc[32:64, :], ov[32:64, :], m16)
        nc.vector.stream_shuffle(bc[64:96, :], ov[96:P, :], m16)
        nc.vector.stream_shuffle(bc[96:P, :], ov[96:P, :], m16)
        nc.vector.reciprocal_approx_fast(rec[:], bc[:])
        nc.vector.tensor_mul(oh[:], ov[:], rec[:])
        return oh

    # p-outer: pair p runs both q-halves back to back (prefetch for pair
    # p+1 spread evenly across both), oh tiles for all pairs/q-halves are
    # kept, and the output projection for each q-half accumulates the 4
    # pairs in PSUM.  outproj(qh0) interleaves into block (p3, qh1)'s fire
    # slots; outproj(qh1) is the tail.
    oh_tiles = {}
    Copy = mybir.ActivationFunctionType.Copy

    def emit_outproj_chunk(qh, e):
        # qs-inner: each pair's weight LDWEIGHTS serves both 512-col q
        # slices; the two PSUM tiles accumulate the 4 pairs concurrently.
        ot = outsb.tile([P, QH], F32, name="ot")
        for qs in range(QH // QB):
            pj = pjp.tile([P, QB], F32, name="pj")
            for p in range(NPAIR):
                nc.tensor.matmul(
                    pj[:],
                    lhsT=wp_sb[:, p, e, :],
                    rhs=oh_tiles[(p, qh)][:, qs * QB:(qs + 1) * QB],
                    start=(p == 0),
                    stop=(p == NPAIR - 1),
                )
            # split the evictions across the two elementwise engines
            if qs == 0:
                nc.scalar.activation(ot[:, 0:QB], pj[:], Copy)
            else:
                nc.vector.tensor_copy(ot[:, QB:QH], pj[:])
        nc.sync.dma_start(out=outt[e, :, qh * QH:(qh + 1) * QH], in_=ot[:])

    def warm_wp():
        junkp = pjp.tile([P, QB], F32, name="pj")
        nc.tensor.matmul(junkp[0:1, 0:1], lhsT=wp_sb[0:1, 0, 0, 0:1],
                         rhs=wp_sb[0:1, 0, 0, 0:1], start=True, stop=True)

    fire_lists[3][1] = [warm_wp] + [
        lambda e=e: emit_outproj_chunk(0, e) for e in range(CC)
    ]

    for p in range(NPAIR):
        for qh in range(2):
            oh_tiles[(p, qh)] = attention_block(p, qh, fire_lists[p][qh])
            assert not fire_lists[p][qh], f"prefetch overflow {p},{qh}"

    for e in range(CC):
        emit_outproj_chunk(1, e)


def _build_program():
    # Bacc (not plain Bass): its compile pipeline legalizes semaphore waits
    # (move_matmul_waits_to_ldweights / generate_event_semaphores) for the
    # 1-wait-per-instruction TRN2 constraint walrus enforces.
    nc = bacc.Bacc(None)
    xt = nc.dram_tensor("xt", [P, CC, N], BF16, kind="ExternalInput")
    wq = nc.dram_tensor("wq", [P, NPAIR, CC, P], BF16, kind="ExternalInput")
    wk = nc.dram_tensor("wk", [P, NPAIR, CC, P], BF16, kind="ExternalInput")
    wv = nc.dram_tensor("wv", [P, CC, 8 * DH], BF16, kind="ExternalInput")
    wp = nc.dram_tensor("wp", [P, NPAIR, CC, P], BF16, kind="ExternalInput")
    bq = nc.dram_tensor("bq", [P, NPAIR], F32, kind="ExternalInput")
    bk = nc.dram_tensor("bk", [P, NPAIR], F32, kind="ExternalInput")
    bv = nc.dram_tensor("bv", [P, 8, DH], F32, kind="ExternalInput")
    outt = nc.dram_tensor("outt", [CC, P, N], F32, kind="ExternalOutput")
    with tile.TileContext(nc) as tc:
        with ExitStack() as ctx:
            _emit(ctx, tc, xt, wq, wk, wv, wp, bq, bk, bv, outt)
    nc.finalize()
    return nc


def _get_program():
    global _PROGRAM
    if _PROGRAM is None:
        _PROGRAM = _build_program()
    return _PROGRAM


def _bf16(a):
    return np.ascontiguousarray(a.astype(ml_dtypes.bfloat16))


def _pairize_cols(W, hg, scale=1.0):
    """[768, 768] -> [768, 512]: pair p gets head hg*8+2p at cols 0:48 and
    head hg*8+2p+1 at cols 64:112 of its 128-col block; the rest zeros."""
    Wp = np.zeros((D, 512), np.float32)
    for p in range(NPAIR):
        ha = (hg * 8 + 2 * p) * DH
        hb = (hg * 8 + 2 * p + 1) * DH
        Wp[:, p * P:p * P + DH] = W[:, ha:ha + DH]
        Wp[:, p * P + 64:p * P + 64 + DH] = W[:, hb:hb + DH]
    if scale != 1.0:
        Wp *= scale
    return Wp


def _pairize_bias(b, hg, scale=1.0):
    bp = np.zeros((P, NPAIR), np.float32)
    for p in range(NPAIR):
        ha = (hg * 8 + 2 * p) * DH
        hb = (hg * 8 + 2 * p + 1) * DH
        bp[0:DH, p] = b[ha:ha + DH]
        bp[64:64 + DH, p] = b[hb:hb + DH]
    if scale != 1.0:
        bp *= scale
    return bp


def _prep_inputs(inputs):
    x = np.asarray(inputs["x"], np.float32)
    WQ = np.asarray(inputs["WQ"], np.float32)
    WK = np.asarray(inputs["WK"], np.float32)
    WV = np.asarray(inputs["WV"], np.float32)
    WP = np.asarray(inputs["WP"], np.float32)
    bQ = np.asarray(inputs["bQ"], np.float32)
    bK = np.asarray(inputs["bK"], np.float32)
    bV = np.asarray(inputs["bV"], np.float32)
    scale = 1.0 / math.sqrt(D)

    per_hg = {}
    for hg in range(2):
        wq_d = _bf16(_pairize_cols(WQ, hg, scale).reshape(CC, P, NPAIR, P).transpose(1, 2, 0, 3))
        wk_d = _bf16(_pairize_cols(WK, hg).reshape(CC, P, NPAIR, P).transpose(1, 2, 0, 3))
        wv_d = _bf16(WV[:, hg * 384:(hg + 1) * 384].reshape(CC, P, 384).transpose(1, 0, 2))
        WPpad = np.zeros((NPAIR, P, D), np.float32)
        for p in range(NPAIR):
            ha = (hg * 8 + 2 * p) * DH
            hb = (hg * 8 + 2 * p + 1) * DH
            WPpad[p, 0:DH] = WP[ha:ha + DH, :]
            WPpad[p, 64:64 + DH] = WP[hb:hb + DH, :]
        wp_d = _bf16(WPpad.reshape(NPAIR, P, CC, P).transpose(1, 0, 2, 3))
        bq_d = _pairize_bias(bQ, hg, scale)
        bk_d = _pairize_bias(bK, hg)
        bv_d = np.ascontiguousarray(
            np.broadcast_to(bV[hg * 384:(hg + 1) * 384].reshape(8, DH), (P, 8, DH))
        )
        per_hg[hg] = dict(wq=wq_d, wk=wk_d, wv=wv_d, wp=wp_d, bq=bq_d, bk=bk_d, bv=bv_d)

    in_maps = []
    for c in range(8):
        b, hg = c // 2, c % 2
        xt_d = _bf16(x[b].T.reshape(CC, P, N).transpose(1, 0, 2))
        m = dict(per_hg[hg])
        m["xt"] = xt_d
        in_maps.append(m)
    return in_maps


def kernel(**inputs):
    global LAST_RESULTS
    bP = np.asarray(inputs["bP"], np.float32)
    nc = _get_program()
    in_maps = _prep_inputs(inputs)
    trace = bool(os.environ.get("BASS_KERNEL_TRACE"))
    tmpdir = os.environ.get("BASS_KERNEL_TMPDIR") or None
    res = run_bass_kernel_spmd(nc, in_maps, list(range(8)), trace=trace, tmpdir=tmpdir)
    LAST_RESULTS = res
    out = np.empty((B, N, D), np.float32)
    for b in range(B):
        t = res.results[2 * b]["outt"].reshape(D, N) + \
            res.results[2 * b + 1]["outt"].reshape(D, N)
        out[b] = t.T + bP
    return out


# revision 35
# speedup vs baseline: 1.2853x; 1.0124x over previous
"""Multi-head attention (B=4, N=2048, D=768, H=16) on 8 trn2 NeuronCores.

Sharding: core c = (batch b = c//2, head-group hg = c%2). Each core computes
attention for 8 heads of one batch element and the partial output projection
for those heads; the host sums the two partial projections per batch (the
tensor-parallel unshard) and adds the output bias.

Device kernel layout (per core) — matmul operands bf16, PSUM/softmax fp32:
  - All projections produce *transposed* activations: QT/KT [dim, seq] so the
    scores matmul S^T = K Q^T needs no transposes; softmax runs along q (free)
    with k on partitions; the row-sum for the softmax denominator is obtained
    by augmenting V with a ones column so the PV matmul emits it for free
    (rows 48/112 of the PSUM accumulator).
  - Heads are processed in pairs packed at partition offsets 0 and 64 so two
    K=48 (QK) / M=64 (PV) matmuls share the PE array via row/col tile groups.
  - exp() is split across engines per head: head A on the scalar engine
    (exact spline exp from PSUM), head B on the vector engine via a one-
    instruction Schraudolph approximation (S*A+B -> int16 whose bits are the
    bf16 exp; ~1.5% rms) so the per-kc softmax latency halves and neither
    engine paces the loop.
  - The PV matmul stream lags the scores stream by one kc so it never waits
    on a fresh exp.
  - Loop order qh-outer / pair-inner: the output projection accumulates the
    4 head-pairs in PSUM via matmul accumulation (no DVE add chain), then one
    DVE copy evicts each chunk for DMA.
  - Normalization reads the PV accumulator straight from PSUM: shuffle-
    broadcast the rowsums, approx-reciprocal, one full-width multiply (pad
    rows are exact zeros because V's pad columns are zero).
  - The next pairs' Q/K projection gens are interleaved into the attention
    loop to fill PE slack; tiny preamble matmuls warm each engine's semaphore
    clocks (walrus allows one wait per lowered instruction).
  - 1/sqrt(768) score scale is folded into WQ (and bQ) on the host.
"""

import math
import os

import numpy as np
import ml_dtypes

import concourse.bass as bass
import concourse.bacc as bacc
import concourse.tile as tile
from concourse import mybir
from concourse.bass_utils import run_bass_kernel_spmd
from contextlib import ExitStack

B, N, D, H, DH = 4, 2048, 768, 16, 48
P = 128
CC = D // P          # 6 contraction chunks of 128
KC = N // P          # 16 key chunks of 128
NPAIR = 4            # head pairs per core (8 heads)
QH = 1024            # q-half width (PSUM budget)
QB = 512             # matmul moving free dim (fp32 max)
F32 = mybir.dt.float32
BF16 = mybir.dt.bfloat16
I16 = mybir.dt.int16

# Schraudolph exp in bf16 bit space: bf16_bits(exp(x)) ~= x*SCH_A + SCH_B
# (then reinterpret the int16 as bf16).  SCH_A = 2^7/ln2; SCH_B centers the
# minimax relative error of the linear-mantissa approximation.
SCH_A = 184.6649652337873
SCH_B = 16245.0

_PROGRAM = None
LAST_RESULTS = None  # BassKernelResults of the most recent run (for test.py)


def _emit(ctx, tc, xt, wq, wk, wv, wp, bq, bk, bv, outt):
    nc = tc.nc
    Exp = mybir.ActivationFunctionType.Exp
    ADD = mybir.AluOpType.add
    MULT = mybir.AluOpType.mult

    consts = ctx.enter_context(tc.tile_pool(name="consts", bufs=1))
    qkvp = ctx.enter_context(tc.tile_pool(name="qkvp", bufs=1))
    vpool = ctx.enter_context(tc.tile_pool(name="vpool", bufs=1))
    ptp = ctx.enter_context(tc.tile_pool(name="ptp", bufs=3))
    ohp = ctx.enter_context(tc.tile_pool(name="ohp", bufs=2))
    rp = ctx.enter_context(tc.tile_pool(name="rp", bufs=1))
    otp = ctx.enter_context(tc.tile_pool(name="otp", bufs=2))
    sp = ctx.enter_context(tc.tile_pool(name="sp", bufs=1, space="PSUM"))
    ovp = ctx.enter_context(tc.tile_pool(name="ovp", bufs=1, space="PSUM"))
    pjp = ctx.enter_context(tc.tile_pool(name="pjp", bufs=2, space="PSUM"))

    # ---- constant loads ----
    wq_sb = consts.tile([P, NPAIR, CC, P], BF16)
    nc.sync.dma_start(out=wq_sb[:], in_=wq[:])
    wk_sb = consts.tile([P, NPAIR, CC, P], BF16)
    nc.sync.dma_start(out=wk_sb[:], in_=wk[:])
    bq_sb = consts.tile([P, NPAIR], F32)
    nc.sync.dma_start(out=bq_sb[:], in_=bq[:])
    bk_sb = consts.tile([P, NPAIR], F32)
    nc.sync.dma_start(out=bk_sb[:], in_=bk[:])
    xt_sb = consts.tile([P, CC, N], BF16)
    for h2 in range(2):
        for c in range(CC):
            nc.sync.dma_start(out=xt_sb[:, c, h2 * (N // 2):(h2 + 1) * (N // 2)],
                              in_=xt[:, c, h2 * (N // 2):(h2 + 1) * (N // 2)])
    wv_sb = consts.tile([P, CC, 8 * DH], BF16)
    nc.sync.dma_start(out=wv_sb[:], in_=wv[:])
    bv_sb = consts.tile([P, 8, DH], F32)
    nc.sync.dma_start(out=bv_sb[:], in_=bv[:])
    wp_sb = consts.tile([P, NPAIR, CC, P], BF16)
    nc.sync.dma_start(out=wp_sb[:], in_=wp[:])

    # ---- engine-clock warm-up ----
    # A self-loading fp32 matmul carries at most ONE semaphore wait in its
    # lowered form, so no real matmul may be the first observer of two new
    # semaphores.  Touch every DMA-loaded operand with a tiny dummy matmul
    # (PE) / copy (DVE) so each engine observes every DMA queue's semaphore
    # before real work begins.
    junk = pjp.tile([P, QB], F32, name="pj")
    for wi, ap in enumerate((
        wq_sb[0:1, 0, 0, 0:1], wk_sb[0:1, 0, 0, 0:1],
        xt_sb[0:1, 0, 0:1], xt_sb[0:1, 1, 0:1], xt_sb[0:1, 2, 0:1],
        xt_sb[0:1, 3, 0:1], xt_sb[0:1, 4, 0:1], xt_sb[0:1, 5, 0:1],
    )):
        nc.tensor.matmul(junk[0:1, wi:wi + 1], lhsT=ap, rhs=ap,
                         start=True, stop=True)
    # wv/wp warm-ups are deferred to just before their first real use so the
    # first QK-projection matmuls don't wait on the whole constant load.
    scr = otp.tile([P, 4], F32, name="scr")
    nc.vector.tensor_copy(scr[0:1, 0:1], bq_sb[0:1, 0:1])
    nc.vector.tensor_copy(scr[0:1, 1:2], bk_sb[0:1, 0:1])
    nc.vector.tensor_copy(scr[0:1, 2:3], bv_sb[0:1, 0, 0:1])

    # ---- Q/K projections (pair-packed transposed layout [128, 2048]) ----
    qt_all = [qkvp.tile([P, N], BF16, name=f"qt{p}") for p in range(NPAIR)]
    kt_all = [qkvp.tile([P, N], BF16, name=f"kt{p}") for p in range(NPAIR)]

    def emit_qk_gen(w_sb, b_sb, dst, pr, qb):
        ps = pjp.tile([P, QB], F32, name="pj")
        for c in range(CC):
            nc.tensor.matmul(
                ps[:],
                lhsT=w_sb[:, pr, c, :],
                rhs=xt_sb[:, c, qb * QB:(qb + 1) * QB],
                start=(c == 0),
                stop=(c == CC - 1),
            )
        nc.vector.tensor_scalar_add(
            dst[:, qb * QB:(qb + 1) * QB], ps[:], b_sb[:, pr:pr + 1]
        )

    # ---- V projection: [k-part, k-chunk, head, 64]: 48 dims | ones | zeros.
    # The ones column makes the PV matmul emit softmax row-sums at psum row
    # 48/112 for free; the zero pad makes PV write exact zeros to the pad
    # rows, so the normalization multiply can sweep all 128 rows.
    v_sb = vpool.tile([P, KC, 8, 64], BF16)
    nc.vector.memset(v_sb[:, :, :, DH:DH + 1], 1.0)
    nc.vector.memset(v_sb[:, :, :, DH + 1:64], 0.0)

    def emit_v_gen(s):
        ps = pjp.tile([P, QB], F32, name="pj")
        for c in range(CC):
            nc.tensor.matmul(
                ps[:, 0:8 * DH],
                lhsT=xt_sb[:, c, s * P:(s + 1) * P],
                rhs=wv_sb[:, c, :],
                start=(c == 0),
                stop=(c == CC - 1),
            )
        nc.vector.scalar_tensor_tensor(
            out=v_sb[:, s, :, 0:DH],
            in0=ps[:, 0:8 * DH].rearrange("p (h d) -> p h d", h=8),
            scalar=1.0,
            in1=bv_sb[:],
            op0=MULT,
            op1=ADD,
        )

    # pair 0 upfront; V chunks 0-3 upfront; the rest interleave into the
    # attention loop (fired from the per-pair prefetch lists) to fill PE
    # slack and keep the matmul-stream density smooth (the board's power
    # governor duty-cycles the PE when density stays high too long).
    for qb in range(4):
        emit_qk_gen(wq_sb, bq_sb, qt_all[0], 0, qb)
    for qb in range(4):
        emit_qk_gen(wk_sb, bk_sb, kt_all[0], 0, qb)

    def warm_wv():
        junkv = pjp.tile([P, QB], F32, name="pj")
        nc.tensor.matmul(junkv[0:1, 0:1], lhsT=wv_sb[0:1, 0, 0:1],
                         rhs=wv_sb[0:1, 0, 0:1], start=True, stop=True)

    warm_wv()
    for s in range(4):
        emit_v_gen(s)

    # prefetch schedule: pair p's two q-half blocks fire the remaining V
    # chunks (p0 only) then pair p+1's QK gens, ~1 per kc iteration.
    fire_lists = {p: {0: [], 1: []} for p in range(NPAIR)}
    for s in range(4, KC):
        fire_lists[0][0].append(lambda s=s: emit_v_gen(s))
    for pr in range(1, NPAIR):
        items = []
        for w_sb, b_sb, dsts in ((wq_sb, bq_sb, qt_all), (wk_sb, bk_sb, kt_all)):
            for qb in range(4):
                items.append(
                    lambda w=w_sb, b=b_sb, d=dsts[pr], pr=pr, qb=qb:
                        emit_qk_gen(w, b, d, pr, qb)
                )
        if pr == 1:
            # p0-qh0 is full with the V gens; all of pair 1's QK goes to qh1
            fire_lists[0][1] += items
        else:
            fire_lists[pr - 1][0] += items[:4]
            fire_lists[pr - 1][1] += items[4:]

    # persistent output staging [128, CC, N] fp32 is too big alongside pt;
    # stage per (e, qh) chunks and DMA immediately.
    outsb = ctx.enter_context(tc.tile_pool(name="outsb", bufs=4))

    m16 = [16] * 32

    def attention_block(p, qh, fires):
        """scores/exp/PV for head-pair p over q-half qh; returns oh tile."""
        q0 = qh * QH
        qt_cur, kt_cur = qt_all[p], kt_all[p]
        ov = ovp.tile([P, QH], F32, name="ov")
        # absorb fresh-tensor semaphore ticks (first observation of the DVE
        # eviction semaphores for this pair's qt/kt) into PE's clock
        junk2 = pjp.tile([P, QB], F32, name="pj")
        nc.tensor.matmul(junk2[0:1, 0:1], lhsT=qt_cur[0:1, 0:1],
                         rhs=qt_cur[0:1, 0:1], start=True, stop=True)
        nc.tensor.matmul(junk2[0:1, 1:2], lhsT=kt_cur[0:1, 0:1],
                         rhs=kt_cur[0:1, 0:1], start=True, stop=True)

        pts = {}

        def emit_scores(i):
            for hh in range(2):
                row0 = 64 * hh
                s_ps = sp.tile([P, QH], F32, name=f"s{hh}")
                for j in range(QH // QB):
                    nc.tensor.matmul(
                        s_ps[:, j * QB:(j + 1) * QB],
                        lhsT=kt_cur[row0:row0 + DH, i * P:(i + 1) * P],
                        rhs=qt_cur[row0:row0 + DH, q0 + j * QB:q0 + (j + 1) * QB],
                        start=True,
                        stop=True,
                        tile_position=(row0, 0),
                    )
                # exp split across engines: DVE takes head B on even kc via
                # the one-instruction Schraudolph approximation (~25% of all
                # scores), ACT the rest with the exact spline exp.
                if (hh == 1 and i % 4 == 0) or (hh == 0 and i % 4 == 2):
                    pt = ptp.tile([P, QH], I16, name="pt1")
                    nc.vector.tensor_scalar(
                        out=pt[:], in0=s_ps[:],
                        scalar1=SCH_A, scalar2=SCH_B,
                        op0=MULT, op1=ADD,
                    )
                else:
                    pt = ptp.tile([P, QH], BF16, name=f"pt0{hh}")
                    nc.scalar.activation(pt[:], s_ps[:], Exp)
                pts[(i, hh)] = pt

        def emit_pv(i):
            for hh in range(2):
                row0 = 64 * hh
                pt = pts.pop((i, hh))
                for j in range(QH // QB):
                    rhs = pt[:, j * QB:(j + 1) * QB]
                    if rhs.dtype == I16:
                        rhs = rhs.bitcast(BF16)
                    nc.tensor.matmul(
                        ov[row0:row0 + 64, j * QB:(j + 1) * QB],
                        lhsT=v_sb[:, i, 2 * p + hh, :],
                        rhs=rhs,
                        start=(i == 0),
                        stop=(i == KC - 1),
                        tile_position=(0, row0),
                        skip_group_check=True,
                    )

        # scores stream runs one kc ahead of the PV stream so PV never
        # waits on a fresh exp.
        emit_scores(0)
        for i in range(1, KC):
            emit_scores(i)
            emit_pv(i - 1)
            if fires:
                fires.pop(0)()
        emit_pv(KC - 1)

        # ---- normalization straight out of PSUM ----
        # rowsums sit at partition 48 (head A) / 112 (head B); shuffle-
        # broadcast them across each 64-row half, approx-reciprocal, then one
        # full-width multiply (pad rows of ov are exact zeros).
        oh = ohp.tile([P, QH], BF16, name=f"oh{p}")
        bc = rp.tile([P, QH], F32, name="bc")
        rec = rp.tile([P, QH], F32, name="rec")
        nc.vector.stream_shuffle(bc[0:32, :], ov[32:64, :], m16)
        nc.vector.stream_shuffle(bc[32:64, :], ov[32:64, :], m16)
        nc.vector.stream_shuffle(bc[64:96, :], ov[96:P, :], m16)
        nc.vector.stream_shuffle(bc[96:P, :], ov[96:P, :], m16)
        nc.vector.reciprocal_approx_fast(rec[:], bc[:])
        nc.vector.tensor_mul(oh[:], ov[:], rec[:])
        return oh

    # p-outer: pair p runs both q-halves back to back (prefetch for pair
    # p+1 spread evenly across both), oh tiles for all pairs/q-halves are
    # kept, and the output projection for each q-half accumulates the 4
    # pairs in PSUM.  outproj(qh0) interleaves into block (p3, qh1)'s fire
    # slots; outproj(qh1) is the tail.
    oh_tiles = {}
    Copy = mybir.ActivationFunctionType.Copy

    def emit_outproj_chunk(qh, e):
        # qs-inner: each pair's weight LDWEIGHTS serves both 512-col q
        # slices; the two PSUM tiles accumulate the 4 pairs concurrently.
        ot = outsb.tile([P, QH], F32, name="ot")
        for qs in range(QH // QB):
            pj = pjp.tile([P, QB], F32, name="pj")
            for p in range(NPAIR):
                nc.tensor.matmul(
                    pj[:],
                    lhsT=wp_sb[:, p, e, :],
                    rhs=oh_tiles[(p, qh)][:, qs * QB:(qs + 1) * QB],
                    start=(p == 0),
                    stop=(p == NPAIR - 1),
                )
            # split the evictions across the two elementwise engines
            if qs == 0:
                nc.scalar.activation(ot[:, 0:QB], pj[:], Copy)
            else:
                nc.vector.tensor_copy(ot[:, QB:QH], pj[:])
        nc.sync.dma_start(out=outt[e, :, qh * QH:(qh + 1) * QH], in_=ot[:])

    def warm_wp():
        junkp = pjp.tile([P, QB], F32, name="pj")
        nc.tensor.matmul(junkp[0:1, 0:1], lhsT=wp_sb[0:1, 0, 0, 0:1],
                         rhs=wp_sb[0:1, 0, 0, 0:1], start=True, stop=True)

    fire_lists[3][1] = [warm_wp] + [
        lambda e=e: emit_outproj_chunk(0, e) for e in range(CC)
    ]

    for p in range(NPAIR):
        for qh in range(2):
            oh_tiles[(p, qh)] = attention_block(p, qh, fire_lists[p][qh])
            assert not fire_lists[p][qh], f"prefetch overflow {p},{qh}"

    for e in range(CC):
        emit_outproj_chunk(1, e)


def _build_program():
    # Bacc (not plain Bass): its compile pipeline legalizes semaphore waits
    # (move_matmul_waits_to_ldweights / generate_event_semaphores) for the
    # 1-wait-per-instruction TRN2 constraint walrus enforces.
    nc = bacc.Bacc(None)
    xt = nc.dram_tensor("xt", [P, CC, N], BF16, kind="ExternalInput")
    wq = nc.dram_tensor("wq", [P, NPAIR, CC, P], BF16, kind="ExternalInput")
    wk = nc.dram_tensor("wk", [P, NPAIR, CC, P], BF16, kind="ExternalInput")
    wv = nc.dram_tensor("wv", [P, CC, 8 * DH], BF16, kind="ExternalInput")
    wp = nc.dram_tensor("wp", [P, NPAIR, CC, P], BF16, kind="ExternalInput")
    bq = nc.dram_tensor("bq", [P, NPAIR], F32, kind="ExternalInput")
    bk = nc.dram_tensor("bk", [P, NPAIR], F32, kind="ExternalInput")
    bv = nc.dram_tensor("bv", [P, 8, DH], F32, kind="ExternalInput")
    outt = nc.dram_tensor("outt", [CC, P, N], F32, kind="ExternalOutput")
    with tile.TileContext(nc) as tc:
        with ExitStack() as ctx:
            _emit(ctx, tc, xt, wq, wk, wv, wp, bq, bk, bv, outt)
    nc.finalize()
    return nc


def _get_program():
    global _PROGRAM
    if _PROGRAM is None:
        _PROGRAM = _build_program()
    return _PROGRAM


def _bf16(a):
    return np.ascontiguousarray(a.astype(ml_dtypes.bfloat16))


def _pairize_cols(W, hg, scale=1.0):
    """[768, 768] -> [768, 512]: pair p gets head hg*8+2p at cols 0:48 and
    head hg*8+2p+1 at cols 64:112 of its 128-col block; the rest zeros."""
    Wp = np.zeros((D, 512), np.float32)
    for p in range(NPAIR):
        ha = (hg * 8 + 2 * p) * DH
        hb = (hg * 8 + 2 * p + 1) * DH
        Wp[:, p * P:p * P + DH] = W[:, ha:ha + DH]
        Wp[:, p * P + 64:p * P + 64 + DH] = W[:, hb:hb + DH]
    if scale != 1.0:
        Wp *= scale
    return Wp


def _pairize_bias(b, hg, scale=1.0):
    bp = np.zeros((P, NPAIR), np.float32)
    for p in range(NPAIR):
        ha = (hg * 8 + 2 * p) * DH
        hb = (hg * 8 + 2 * p + 1) * DH
        bp[0:DH, p] = b[ha:ha + DH]
        bp[64:64 + DH, p] = b[hb:hb + DH]
    if scale != 1.0:
        bp *= scale
    return bp


def _prep_inputs(inputs):
    x = np.asarray(inputs["x"], np.float32)
    WQ = np.asarray(inputs["WQ"], np.float32)
    WK = np.asarray(inputs["WK"], np.float32)
    WV = np.asarray(inputs["WV"], np.float32)
    WP = np.asarray(inputs["WP"], np.float32)
    bQ = np.asarray(inputs["bQ"], np.float32)
    bK = np.asarray(inputs["bK"], np.float32)
    bV = np.asarray(inputs["bV"], np.float32)
    scale = 1.0 / math.sqrt(D)

    per_hg = {}
    for hg in range(2):
        wq_d = _bf16(_pairize_cols(WQ, hg, scale).reshape(CC, P, NPAIR, P).transpose(1, 2, 0, 3))
        wk_d = _bf16(_pairize_cols(WK, hg).reshape(CC, P, NPAIR, P).transpose(1, 2, 0, 3))
        wv_d = _bf16(WV[:, hg * 384:(hg + 1) * 384].reshape(CC, P, 384).transpose(1, 0, 2))
        WPpad = np.zeros((NPAIR, P, D), np.float32)
        for p in range(NPAIR):
            ha = (hg * 8 + 2 * p) * DH
            hb = (hg * 8 + 2 * p + 1) * DH
            WPpad[p, 0:DH] = WP[ha:ha + DH, :]
            WPpad[p, 64:64 + DH] = WP[hb:hb + DH, :]
        wp_d = _bf16(WPpad.reshape(NPAIR, P, CC, P).transpose(1, 0, 2, 3))
        bq_d = _pairize_bias(bQ, hg, scale)
        bk_d = _pairize_bias(bK, hg)
        bv_d = np.ascontiguousarray(
            np.broadcast_to(bV[hg * 384:(hg + 1) * 384].reshape(8, DH), (P, 8, DH))
        )
        per_hg[hg] = dict(wq=wq_d, wk=wk_d, wv=wv_d, wp=wp_d, bq=bq_d, bk=bk_d, bv=bv_d)

    in_maps = []
    for c in range(8):
        b, hg = c // 2, c % 2
        xt_d = _bf16(x[b].T.reshape(CC, P, N).transpose(1, 0, 2))
        m = dict(per_hg[hg])
        m["xt"] = xt_d
        in_maps.append(m)
    return in_maps


def kernel(**inputs):
    global LAST_RESULTS
    bP = np.asarray(inputs["bP"], np.float32)
    nc = _get_program()
    in_maps = _prep_inputs(inputs)
    trace = bool(os.environ.get("BASS_KERNEL_TRACE"))
    tmpdir = os.environ.get("BASS_KERNEL_TMPDIR") or None
    res = run_bass_kernel_spmd(nc, in_maps, list(range(8)), trace=trace, tmpdir=tmpdir)
    LAST_RESULTS = res
    out = np.empty((B, N, D), np.float32)
    for b in range(B):
        t = res.results[2 * b]["outt"].reshape(D, N) + \
            res.results[2 * b + 1]["outt"].reshape(D, N)
        out[b] = t.T + bP
    return out


# revision 36
# speedup vs baseline: 1.2933x; 1.0062x over previous
"""Multi-head attention (B=4, N=2048, D=768, H=16) on 8 trn2 NeuronCores.

Sharding: core c = (batch b = c//2, head-group hg = c%2). Each core computes
attention for 8 heads of one batch element and the partial output projection
for those heads; the host sums the two partial projections per batch (the
tensor-parallel unshard) and adds the output bias.

Device kernel layout (per core) — matmul operands bf16, PSUM/softmax fp32:
  - All projections produce *transposed* activations: QT/KT [dim, seq] so the
    scores matmul S^T = K Q^T needs no transposes; softmax runs along q (free)
    with k on partitions; the row-sum for the softmax denominator is obtained
    by augmenting V with a ones column so the PV matmul emits it for free
    (rows 48/112 of the PSUM accumulator).
  - Heads are processed in pairs packed at partition offsets 0 and 64 so two
    K=48 (QK) / M=64 (PV) matmuls share the PE array via row/col tile groups.
  - exp() is split across engines on a regular period-4 schedule: 3/4 of
    the [128,1024] score tiles take the exact spline exp on the scalar
    engine, 1/4 take a one-instruction Schraudolph approximation on the
    vector engine (S*A+B -> int16 whose bits are the bf16 exp; ~1.5% rms).
    The DVE tile alternates between head A and head B so each score
    buffer's reuse chain alternates consumer engines (relaxes the
    write-after-read critical path).  25% keeps the DVE (which also owns
    normalization and projection evictions) level with the scalar engine.
  - The PV matmul stream lags the scores stream by one kc so it never waits
    on a fresh exp.
  - Loop order qh-outer / pair-inner: the output projection accumulates the
    4 head-pairs in PSUM via matmul accumulation (no DVE add chain), then one
    DVE copy evicts each chunk for DMA.
  - Normalization reads the PV accumulator straight from PSUM: shuffle-
    broadcast the rowsums, approx-reciprocal, one full-width multiply (pad
    rows are exact zeros because V's pad columns are zero).
  - The next pairs' Q/K projection gens are interleaved into the attention
    loop to fill PE slack; tiny preamble matmuls warm each engine's semaphore
    clocks (walrus allows one wait per lowered instruction).
  - 1/sqrt(768) score scale is folded into WQ (and bQ) on the host.
"""

import math
import os

import numpy as np
import ml_dtypes

import concourse.bass as bass
import concourse.bacc as bacc
import concourse.tile as tile
from concourse import mybir
from concourse.bass_utils import run_bass_kernel_spmd
from contextlib import ExitStack

B, N, D, H, DH = 4, 2048, 768, 16, 48
P = 128
CC = D // P          # 6 contraction chunks of 128
KC = N // P          # 16 key chunks of 128
NPAIR = 4            # head pairs per core (8 heads)
QH = 1024            # q-half width (PSUM budget)
QB = 512             # matmul moving free dim (fp32 max)
F32 = mybir.dt.float32
BF16 = mybir.dt.bfloat16
I16 = mybir.dt.int16

# Schraudolph exp in bf16 bit space: bf16_bits(exp(x)) ~= x*SCH_A + SCH_B
# (then reinterpret the int16 as bf16).  SCH_A = 2^7/ln2; SCH_B centers the
# minimax relative error of the linear-mantissa approximation.
SCH_A = 184.6649652337873
SCH_B = 16245.0

_PROGRAM = None
LAST_RESULTS = None  # BassKernelResults of the most recent run (for test.py)


def _emit(ctx, tc, xt, wq, wk, wv, wp, bq, bk, bv, outt):
    nc = tc.nc
    Exp = mybir.ActivationFunctionType.Exp
    ADD = mybir.AluOpType.add
    MULT = mybir.AluOpType.mult

    consts = ctx.enter_context(tc.tile_pool(name="consts", bufs=1))
    qkvp = ctx.enter_context(tc.tile_pool(name="qkvp", bufs=1))
    vpool = ctx.enter_context(tc.tile_pool(name="vpool", bufs=1))
    ptp = ctx.enter_context(tc.tile_pool(name="ptp", bufs=3))
    ohp = ctx.enter_context(tc.tile_pool(name="ohp", bufs=2))
    rp = ctx.enter_context(tc.tile_pool(name="rp", bufs=1))
    otp = ctx.enter_context(tc.tile_pool(name="otp", bufs=2))
    sp = ctx.enter_context(tc.tile_pool(name="sp", bufs=1, space="PSUM"))
    ovp = ctx.enter_context(tc.tile_pool(name="ovp", bufs=1, space="PSUM"))
    pjp = ctx.enter_context(tc.tile_pool(name="pjp", bufs=2, space="PSUM"))

    # ---- constant loads ----
    wq_sb = consts.tile([P, NPAIR, CC, P], BF16)
    nc.sync.dma_start(out=wq_sb[:], in_=wq[:])
    wk_sb = consts.tile([P, NPAIR, CC, P], BF16)
    nc.sync.dma_start(out=wk_sb[:], in_=wk[:])
    bq_sb = consts.tile([P, NPAIR], F32)
    nc.sync.dma_start(out=bq_sb[:], in_=bq[:])
    bk_sb = consts.tile([P, NPAIR], F32)
    nc.sync.dma_start(out=bk_sb[:], in_=bk[:])
    xt_sb = consts.tile([P, CC, N], BF16)
    for h2 in range(2):
        for c in range(CC):
            nc.sync.dma_start(out=xt_sb[:, c, h2 * (N // 2):(h2 + 1) * (N // 2)],
                              in_=xt[:, c, h2 * (N // 2):(h2 + 1) * (N // 2)])
    wv_sb = consts.tile([P, CC, 8 * DH], BF16)
    nc.sync.dma_start(out=wv_sb[:], in_=wv[:])
    bv_sb = consts.tile([P, 8, DH], F32)
    nc.sync.dma_start(out=bv_sb[:], in_=bv[:])
    wp_sb = consts.tile([P, NPAIR, CC, P], BF16)
    nc.sync.dma_start(out=wp_sb[:], in_=wp[:])

    # ---- engine-clock warm-up ----
    # A self-loading fp32 matmul carries at most ONE semaphore wait in its
    # lowered form, so no real matmul may be the first observer of two new
    # semaphores.  Touch every DMA-loaded operand with a tiny dummy matmul
    # (PE) / copy (DVE) so each engine observes every DMA queue's semaphore
    # before real work begins.
    junk = pjp.tile([P, QB], F32, name="pj")
    for wi, ap in enumerate((
        wq_sb[0:1, 0, 0, 0:1], wk_sb[0:1, 0, 0, 0:1],
        xt_sb[0:1, 0, 0:1], xt_sb[0:1, 1, 0:1], xt_sb[0:1, 2, 0:1],
        xt_sb[0:1, 3, 0:1], xt_sb[0:1, 4, 0:1], xt_sb[0:1, 5, 0:1],
    )):
        nc.tensor.matmul(junk[0:1, wi:wi + 1], lhsT=ap, rhs=ap,
                         start=True, stop=True)
    # wv/wp warm-ups are deferred to just before their first real use so the
    # first QK-projection matmuls don't wait on the whole constant load.
    scr = otp.tile([P, 4], F32, name="scr")
    nc.vector.tensor_copy(scr[0:1, 0:1], bq_sb[0:1, 0:1])
    nc.vector.tensor_copy(scr[0:1, 1:2], bk_sb[0:1, 0:1])
    nc.vector.tensor_copy(scr[0:1, 2:3], bv_sb[0:1, 0, 0:1])

    # ---- Q/K projections (pair-packed transposed layout [128, 2048]) ----
    qt_all = [qkvp.tile([P, N], BF16, name=f"qt{p}") for p in range(NPAIR)]
    kt_all = [qkvp.tile([P, N], BF16, name=f"kt{p}") for p in range(NPAIR)]

    def emit_qk_gen(w_sb, b_sb, dst, pr, qb):
        ps = pjp.tile([P, QB], F32, name="pj")
        for c in range(CC):
            nc.tensor.matmul(
                ps[:],
                lhsT=w_sb[:, pr, c, :],
                rhs=xt_sb[:, c, qb * QB:(qb + 1) * QB],
                start=(c == 0),
                stop=(c == CC - 1),
            )
        nc.vector.tensor_scalar_add(
            dst[:, qb * QB:(qb + 1) * QB], ps[:], b_sb[:, pr:pr + 1]
        )

    # ---- V projection: [k-part, k-chunk, head, 64]: 48 dims | ones | zeros.
    # The ones column makes the PV matmul emit softmax row-sums at psum row
    # 48/112 for free; the zero pad makes PV write exact zeros to the pad
    # rows, so the normalization multiply can sweep all 128 rows.
    v_sb = vpool.tile([P, KC, 8, 64], BF16)
    nc.vector.memset(v_sb[:, :, :, DH:DH + 1], 1.0)
    nc.vector.memset(v_sb[:, :, :, DH + 1:64], 0.0)

    def emit_v_gen(s):
        ps = pjp.tile([P, QB], F32, name="pj")
        for c in range(CC):
            nc.tensor.matmul(
                ps[:, 0:8 * DH],
                lhsT=xt_sb[:, c, s * P:(s + 1) * P],
                rhs=wv_sb[:, c, :],
                start=(c == 0),
                stop=(c == CC - 1),
            )
        nc.vector.scalar_tensor_tensor(
            out=v_sb[:, s, :, 0:DH],
            in0=ps[:, 0:8 * DH].rearrange("p (h d) -> p h d", h=8),
            scalar=1.0,
            in1=bv_sb[:],
            op0=MULT,
            op1=ADD,
        )

    # pair 0 upfront; V chunks 0-3 upfront; the rest interleave into the
    # attention loop (fired from the per-pair prefetch lists) to fill PE
    # slack and keep the matmul-stream density smooth (the board's power
    # governor duty-cycles the PE when density stays high too long).
    for qb in range(4):
        emit_qk_gen(wq_sb, bq_sb, qt_all[0], 0, qb)
    for qb in range(4):
        emit_qk_gen(wk_sb, bk_sb, kt_all[0], 0, qb)

    def warm_wv():
        junkv = pjp.tile([P, QB], F32, name="pj")
        nc.tensor.matmul(junkv[0:1, 0:1], lhsT=wv_sb[0:1, 0, 0:1],
                         rhs=wv_sb[0:1, 0, 0:1], start=True, stop=True)

    warm_wv()
    for s in range(4):
        emit_v_gen(s)

    # prefetch schedule: pair p's two q-half blocks fire the remaining V
    # chunks (p0 only) then pair p+1's QK gens, ~1 per kc iteration.
    fire_lists = {p: {0: [], 1: []} for p in range(NPAIR)}
    for s in range(4, KC):
        fire_lists[0][0].append(lambda s=s: emit_v_gen(s))
    for pr in range(1, NPAIR):
        items = []
        for w_sb, b_sb, dsts in ((wq_sb, bq_sb, qt_all), (wk_sb, bk_sb, kt_all)):
            for qb in range(4):
                items.append(
                    lambda w=w_sb, b=b_sb, d=dsts[pr], pr=pr, qb=qb:
                        emit_qk_gen(w, b, d, pr, qb)
                )
        if pr == 1:
            # p0-qh0 is full with the V gens; all of pair 1's QK goes to qh1
            fire_lists[0][1] += items
        else:
            fire_lists[pr - 1][0] += items[:4]
            fire_lists[pr - 1][1] += items[4:]

    # persistent output staging [128, CC, N] fp32 is too big alongside pt;
    # stage per (e, qh) chunks and DMA immediately.
    outsb = ctx.enter_context(tc.tile_pool(name="outsb", bufs=4))

    m16 = [16] * 32

    def attention_block(p, qh, fires):
        """scores/exp/PV for head-pair p over q-half qh; returns oh tile."""
        q0 = qh * QH
        qt_cur, kt_cur = qt_all[p], kt_all[p]
        ov = ovp.tile([P, QH], F32, name="ov")
        # absorb fresh-tensor semaphore ticks (first observation of the DVE
        # eviction semaphores for this pair's qt/kt) into PE's clock
        junk2 = pjp.tile([P, QB], F32, name="pj")
        nc.tensor.matmul(junk2[0:1, 0:1], lhsT=qt_cur[0:1, 0:1],
                         rhs=qt_cur[0:1, 0:1], start=True, stop=True)
        nc.tensor.matmul(junk2[0:1, 1:2], lhsT=kt_cur[0:1, 0:1],
                         rhs=kt_cur[0:1, 0:1], start=True, stop=True)

        pts = {}

        def emit_scores(i):
            for hh in range(2):
                row0 = 64 * hh
                s_ps = sp.tile([P, QH], F32, name=f"s{hh}")
                for j in range(QH // QB):
                    nc.tensor.matmul(
                        s_ps[:, j * QB:(j + 1) * QB],
                        lhsT=kt_cur[row0:row0 + DH, i * P:(i + 1) * P],
                        rhs=qt_cur[row0:row0 + DH, q0 + j * QB:q0 + (j + 1) * QB],
                        start=True,
                        stop=True,
                        tile_position=(row0, 0),
                    )
                # exp split across engines: DVE takes head B on even kc via
                # the one-instruction Schraudolph approximation (~25% of all
                # scores), ACT the rest with the exact spline exp.
                if (hh == 1 and i % 4 == 0) or (hh == 0 and i % 4 == 2):
                    pt = ptp.tile([P, QH], I16, name="pt1")
                    nc.vector.tensor_scalar(
                        out=pt[:], in0=s_ps[:],
                        scalar1=SCH_A, scalar2=SCH_B,
                        op0=MULT, op1=ADD,
                    )
                else:
                    pt = ptp.tile([P, QH], BF16, name=f"pt0{hh}")
                    nc.scalar.activation(pt[:], s_ps[:], Exp)
                pts[(i, hh)] = pt

        def emit_pv(i):
            for hh in range(2):
                row0 = 64 * hh
                pt = pts.pop((i, hh))
                for j in range(QH // QB):
                    rhs = pt[:, j * QB:(j + 1) * QB]
                    if rhs.dtype == I16:
                        rhs = rhs.bitcast(BF16)
                    nc.tensor.matmul(
                        ov[row0:row0 + 64, j * QB:(j + 1) * QB],
                        lhsT=v_sb[:, i, 2 * p + hh, :],
                        rhs=rhs,
                        start=(i == 0),
                        stop=(i == KC - 1),
                        tile_position=(0, row0),
                        skip_group_check=True,
                    )

        # scores stream runs one kc ahead of the PV stream so PV never
        # waits on a fresh exp.
        emit_scores(0)
        for i in range(1, KC):
            emit_scores(i)
            emit_pv(i - 1)
            if fires:
                fires.pop(0)()
        emit_pv(KC - 1)

        # ---- normalization straight out of PSUM ----
        # rowsums sit at partition 48 (head A) / 112 (head B); shuffle-
        # broadcast them across each 64-row half, approx-reciprocal, then one
        # full-width multiply (pad rows of ov are exact zeros).
        oh = ohp.tile([P, QH], BF16, name=f"oh{p}")
        bc = rp.tile([P, QH], F32, name="bc")
        rec = rp.tile([P, QH], F32, name="rec")
        nc.vector.stream_shuffle(bc[0:32, :], ov[32:64, :], m16)
        nc.vector.stream_shuffle(bc[32:64, :], ov[32:64, :], m16)
        nc.vector.stream_shuffle(bc[64:96, :], ov[96:P, :], m16)
        nc.vector.stream_shuffle(bc[96:P, :], ov[96:P, :], m16)
        nc.vector.reciprocal_approx_fast(rec[:], bc[:])
        nc.vector.tensor_mul(oh[:], ov[:], rec[:])
        return oh

    # p-outer: pair p runs both q-halves back to back (prefetch for pair
    # p+1 spread evenly across both), oh tiles for all pairs/q-halves are
    # kept, and the output projection for each q-half accumulates the 4
    # pairs in PSUM.  outproj(qh0) interleaves into block (p3, qh1)'s fire
    # slots; outproj(qh1) is the tail.
    oh_tiles = {}
    Copy = mybir.ActivationFunctionType.Copy

    def emit_outproj_chunk(qh, e):
        # qs-inner: each pair's weight LDWEIGHTS serves both 512-col q
        # slices; the two PSUM tiles accumulate the 4 pairs concurrently.
        ot = outsb.tile([P, QH], F32, name="ot")
        for qs in range(QH // QB):
            pj = pjp.tile([P, QB], F32, name="pj")
            for p in range(NPAIR):
                nc.tensor.matmul(
                    pj[:],
                    lhsT=wp_sb[:, p, e, :],
                    rhs=oh_tiles[(p, qh)][:, qs * QB:(qs + 1) * QB],
                    start=(p == 0),
                    stop=(p == NPAIR - 1),
                )
            # split the evictions across the two elementwise engines
            if qs == 0:
                nc.scalar.activation(ot[:, 0:QB], pj[:], Copy)
            else:
                nc.vector.tensor_copy(ot[:, QB:QH], pj[:])
        nc.sync.dma_start(out=outt[e, :, qh * QH:(qh + 1) * QH], in_=ot[:])

    def warm_wp():
        junkp = pjp.tile([P, QB], F32, name="pj")
        nc.tensor.matmul(junkp[0:1, 0:1], lhsT=wp_sb[0:1, 0, 0, 0:1],
                         rhs=wp_sb[0:1, 0, 0, 0:1], start=True, stop=True)

    fire_lists[3][1] = [warm_wp] + [
        lambda e=e: emit_outproj_chunk(0, e) for e in range(CC)
    ]

    for p in range(NPAIR):
        for qh in range(2):
            oh_tiles[(p, qh)] = attention_block(p, qh, fire_lists[p][qh])
            assert not fire_lists[p][qh], f"prefetch overflow {p},{qh}"

    for e in range(CC):
        emit_outproj_chunk(1, e)


def _build_program():
    # Bacc (not plain Bass): its compile pipeline legalizes semaphore waits
    # (move_matmul_waits_to_ldweights / generate_event_semaphores) for the
    # 1-wait-per-instruction TRN2 constraint walrus enforces.
    nc = bacc.Bacc(None)
    xt = nc.dram_tensor("xt", [P, CC, N], BF16, kind="ExternalInput")
    wq = nc.dram_tensor("wq", [P, NPAIR, CC, P], BF16, kind="ExternalInput")
    wk = nc.dram_tensor("wk", [P, NPAIR, CC, P], BF16, kind="ExternalInput")
    wv = nc.dram_tensor("wv", [P, CC, 8 * DH], BF16, kind="ExternalInput")
    wp = nc.dram_tensor("wp", [P, NPAIR, CC, P], BF16, kind="ExternalInput")
    bq = nc.dram_tensor("bq", [P, NPAIR], F32, kind="ExternalInput")
    bk = nc.dram_tensor("bk", [P, NPAIR], F32, kind="ExternalInput")
    bv = nc.dram_tensor("bv", [P, 8, DH], F32, kind="ExternalInput")
    outt = nc.dram_tensor("outt", [CC, P, N], F32, kind="ExternalOutput")
    with tile.TileContext(nc) as tc:
        with ExitStack() as ctx:
            _emit(ctx, tc, xt, wq, wk, wv, wp, bq, bk, bv, outt)
    nc.finalize()
    return nc


def _get_program():
    global _PROGRAM
    if _PROGRAM is None:
        _PROGRAM = _build_program()
    return _PROGRAM


def _bf16(a):
    return np.ascontiguousarray(a.astype(ml_dtypes.bfloat16))


def _pairize_cols(W, hg, scale=1.0):
    """[768, 768] -> [768, 512]: pair p gets head hg*8+2p at cols 0:48 and
    head hg*8+2p+1 at cols 64:112 of its 128-col block; the rest zeros."""
    Wp = np.zeros((D, 512), np.float32)
    for p in range(NPAIR):
        ha = (hg * 8 + 2 * p) * DH
        hb = (hg * 8 + 2 * p + 1) * DH
        Wp[:, p * P:p * P + DH] = W[:, ha:ha + DH]
        Wp[:, p * P + 64:p * P + 64 + DH] = W[:, hb:hb + DH]
    if scale != 1.0:
        Wp *= scale
    return Wp


def _pairize_bias(b, hg, scale=1.0):
    bp = np.zeros((P, NPAIR), np.float32)
    for p in range(NPAIR):
        ha = (hg * 8 + 2 * p) * DH
        hb = (hg * 8 + 2 * p + 1) * DH
        bp[0:DH, p] = b[ha:ha + DH]
        bp[64:64 + DH, p] = b[hb:hb + DH]
    if scale != 1.0:
        bp *= scale
    return bp


def _prep_inputs(inputs):
    x = np.asarray(inputs["x"], np.float32)
    WQ = np.asarray(inputs["WQ"], np.float32)
    WK = np.asarray(inputs["WK"], np.float32)
    WV = np.asarray(inputs["WV"], np.float32)
    WP = np.asarray(inputs["WP"], np.float32)
    bQ = np.asarray(inputs["bQ"], np.float32)
    bK = np.asarray(inputs["bK"], np.float32)
    bV = np.asarray(inputs["bV"], np.float32)
    scale = 1.0 / math.sqrt(D)

    per_hg = {}
    for hg in range(2):
        wq_d = _bf16(_pairize_cols(WQ, hg, scale).reshape(CC, P, NPAIR, P).transpose(1, 2, 0, 3))
        wk_d = _bf16(_pairize_cols(WK, hg).reshape(CC, P, NPAIR, P).transpose(1, 2, 0, 3))
        wv_d = _bf16(WV[:, hg * 384:(hg + 1) * 384].reshape(CC, P, 384).transpose(1, 0, 2))
        WPpad = np.zeros((NPAIR, P, D), np.float32)
        for p in range(NPAIR):
            ha = (hg * 8 + 2 * p) * DH
            hb = (hg * 8 + 2 * p + 1) * DH
            WPpad[p, 0:DH] = WP[ha:ha + DH, :]
            WPpad[p, 64:64 + DH] = WP[hb:hb + DH, :]
        wp_d = _bf16(WPpad.reshape(NPAIR, P, CC, P).transpose(1, 0, 2, 3))
        bq_d = _pairize_bias(bQ, hg, scale)
        bk_d = _pairize_bias(bK, hg)
        bv_d = np.ascontiguousarray(
            np.broadcast_to(bV[hg * 384:(hg + 1) * 384].reshape(8, DH), (P, 8, DH))
        )
        per_hg[hg] = dict(wq=wq_d, wk=wk_d, wv=wv_d, wp=wp_d, bq=bq_d, bk=bk_d, bv=bv_d)

    in_maps = []
    for c in range(8):
        b, hg = c // 2, c % 2
        xt_d = _bf16(x[b].T.reshape(CC, P, N).transpose(1, 0, 2))
        m = dict(per_hg[hg])
        m["xt"] = xt_d
        in_maps.append(m)
    return in_maps


def kernel(**inputs):
    global LAST_RESULTS
    bP = np.asarray(inputs["bP"], np.float32)
    nc = _get_program()
    in_maps = _prep_inputs(inputs)
    trace = bool(os.environ.get("BASS_KERNEL_TRACE"))
    tmpdir = os.environ.get("BASS_KERNEL_TMPDIR") or None
    res = run_bass_kernel_spmd(nc, in_maps, list(range(8)), trace=trace, tmpdir=tmpdir)
    LAST_RESULTS = res
    out = np.empty((B, N, D), np.float32)
    for b in range(B):
        t = res.results[2 * b]["outt"].reshape(D, N) + \
            res.results[2 * b + 1]["outt"].reshape(D, N)
        out[b] = t.T + bP
    return out
